# revision 43
# baseline (speedup 1.0000x reference)
"""3-layer GAT (8 heads x 64 ch) + global mean pool + FC + log_softmax on 8 Trainium2 cores.

v2 redesign of the edge phase around batched per-tile operations (vs the
per-column baseline):
- Nodes (and their incoming edges) partitioned across 8 cores; small GAT
  weights replicated. Per layer each core computes [h | a_src] rows (bf16,
  channel-major) for its node shard and AllGathers them into a replicated
  DRAM table; the graph-level mean pool is the only AllReduce.
- Per 128-dst-node tile: ONE wide indirect DMA gathers all edge source rows
  (multi-column offset AP amortizes the ~1us SWDGE descriptor-generation
  cost ~10x), and the attention math runs as a handful of wide strided-AP
  ops. Channel-major rows put the 8 heads innermost so the alpha broadcast
  views keep packed innermost strides (DVE 2x bf16 mode) for the big
  h*alpha multiply.
- alpha is pre-normalized (ex * 1/den) before the message multiply; for
  one-hot overflow chunks the per-edge denominator is fetched through the
  same one-hot matmul used for a_dst. This removes the per-tile
  divide-from-PSUM and lets the layer bias fold into the PSUM accumulation
  as an identity-matmul of replicated bias rows, so the finalize is a
  single Relu activation straight out of PSUM.
- Engine placement: Pool runs the gathers + SBUF-only logit adds; DVE the
  one-hot builds, denominators and the msg multiply; Act exp/copies/relu;
  PE transposes, scatter-add and the fused next-layer x@W (stage A), which
  hides under the gather-DMA phase. bf16 matmuls run at 1 cycle/row (fp32
  is 4).
- `repeat=` builds the body K times (fresh DRAM tables per rep) purely for
  timing: (T(K)-T(1))/(K-1) cancels the ~80ms axon RPC floor.
"""

import numpy as np
from ml_dtypes import bfloat16 as np_bf16

import concourse.bass as bass
import concourse.mybir as mybir
import concourse.tile as tile
from concourse import bacc
from concourse.bass_utils import run_bass_kernel_spmd
from concourse.masks import make_identity

# problem constants (hardcoded per contract)
N, E, F_IN, H, C, G, NCLS = 50000, 400000, 128, 8, 64, 64, 10
HC = H * C  # 512
NEG = 0.2
EPS = 1e-16

NCORES = 8
P = 128
NSH = N // NCORES          # 6250 nodes per core
NT = (NSH + P - 1) // P    # 49 dst tiles per core
NSHP = NT * P              # 6272 padded rows per core
NFULL = NCORES * NSHP      # 50176 rows in the gathered table
ROWW = HC + H              # 520: h (chan-major) | a_src
WEXT = HC + 2 * H          # 528: W | Wa_src | Wa_dst
PAD = 999.0                # one-hot miss marker for padded overflow slots
MASKNEG = -1.0e30          # a_src value planted in the pad row (=> exp -> 0)
CCAP = 4                   # max one-hot chunk columns per tile (PSUM bank)
NCAP = 16                  # max total columns per tile (SBUF budget)
# hx AllGather split boundaries (tile granularity). A Shared DRAM tensor
# may only be written by one instruction, so the hx AG is a single segment.
SPLIT_TILES = (0, 49)
SPLIT_ROWS = tuple(t * P for t in SPLIT_TILES)
# pad slots gather the pad row (core 0, local row NSHP-1) whose a_src holds
# MASKNEG, so padded round edges contribute exp(-inf)=0 without a mask op.
PADGID_H = NCORES * SPLIT_ROWS[-2] + (NSHP - 1 - SPLIT_ROWS[-2])
PADGID_A = NSHP - 1
PADSTART = NSH - (NT - 1) * P  # first pad partition in the last tile
GRP = 7                    # alpha-loop stage-major group size (tiles)

F32 = mybir.dt.float32
BF16 = mybir.dt.bfloat16
I32 = mybir.dt.int32


def _preprocess_edges(edge_index):
    """Assign edges (incl. self loops) to the dst-owning core; build per-tile
    round columns (node's r-th edge in its own partition) plus overflow
    one-hot chunk columns.

    Returns (plan, esrcT, edstT):
      plan: list of (R_t, C_t) per tile — shared by all cores.
      esrcT[k] int32 [P, TOTCOL]: hx gather row ids, seg-major per
        SPLIT_ROWS (pad -> PADGID_H, whose a_src holds MASKNEG).
      edstT[k] f32 [P, TOTCOL]: chunk cols -> dst slot or PAD (round cols
        unused by the kernel)."""
    src = np.concatenate([edge_index[0], np.arange(N, dtype=np.int64)])
    dst = np.concatenate([edge_index[1], np.arange(N, dtype=np.int64)])
    core = dst // NSH
    dloc = dst - core * NSH
    tile_of = dloc // P
    slot = dloc - tile_of * P
    # hx gather table layout: SPLITS segments, each rank-major over its rows
    sk = src // NSH
    sr = src % NSH
    split_rows = np.asarray(SPLIT_ROWS, np.int64)
    seg = np.searchsorted(split_rows[1:], sr, side="right")
    r0 = split_rows[seg]
    r1 = split_rows[seg + 1]
    gid = NCORES * r0 + sk * (r1 - r0) + (sr - r0)

    deg = np.zeros((NCORES, NT, P), np.int64)
    np.add.at(deg, (core, tile_of, slot), 1)
    maxdeg_t = deg.max(axis=(0, 2))  # [NT]

    # choose the round cap per tile: chunk columns carry extra one-hot work
    R_ts = np.zeros(NT, np.int64)
    C_ts = np.zeros(NT, np.int64)
    for t in range(NT):
        best = None
        for rcap in range(1, int(maxdeg_t[t]) + 1):
            r = min(int(maxdeg_t[t]), rcap)
            ovf = np.maximum(deg[:, t, :] - rcap, 0).sum(axis=1)
            c = int(np.ceil(ovf / P).max())
            if c > CCAP or r + c > NCAP:
                continue
            cost = r + 1.35 * c
            if best is None or cost < best[0]:
                best = (cost, r, c)
        assert best is not None, f"no feasible plan for tile {t}"
        _, R_ts[t], C_ts[t] = best
    plan = [(int(R_ts[t]), int(C_ts[t])) for t in range(NT)]
    colbase = np.zeros(NT, np.int64)
    colbase[1:] = np.cumsum(R_ts + C_ts)[:-1]
    TOTCOL = int((R_ts + C_ts).sum())

    esrcT = np.full((NCORES, P, TOTCOL), PADGID_H, np.int32)
    edstT = np.empty((NCORES, P, TOTCOL), np.float32)
    for k in range(NCORES):
        # default fill: chunks PAD (rounds unused)
        for t in range(NT):
            b = colbase[t]
            edstT[k, :, b:b + C_ts[t]] = PAD
            edstT[k, :, b + C_ts[t]:b + C_ts[t] + R_ts[t]] = 0.0
        m = core == k
        t_k, s_k, g_k = tile_of[m], slot[m], gid[m]
        order = np.argsort(t_k * P + s_k, kind="stable")
        t_k, s_k, g_k = t_k[order], s_k[order], g_k[order]
        node = t_k * P + s_k
        start = np.zeros(NT * P + 1, np.int64)
        np.add.at(start[1:], node, 1)
        start = np.cumsum(start)
        j = np.arange(len(node)) - start[node]  # rank within node
        rmax = R_ts[t_k]
        isr = j < rmax
        # round entries (after the C_t chunk columns)
        rcol = colbase[t_k[isr]] + C_ts[t_k[isr]] + j[isr]
        esrcT[k, s_k[isr], rcol] = g_k[isr].astype(np.int32)
        # overflow entries: sequential position within each tile
        to, so, go = t_k[~isr], s_k[~isr], g_k[~isr]
        oorder = np.argsort(to, kind="stable")
        to, so, go = to[oorder], so[oorder], go[oorder]
        ostart = np.zeros(NT + 1, np.int64)
        np.add.at(ostart[1:], to, 1)
        ostart = np.cumsum(ostart)
        q = np.arange(len(to)) - ostart[to]
        col = colbase[to] + q // P
        row = q % P
        esrcT[k, row, col] = go.astype(np.int32)
        edstT[k, row, col] = so.astype(np.float32)
    return plan, esrcT, edstT


# channel-major permutation: cm index (c*H + h) <- standard (h*C + c)
_PERM_CM = np.array([h * C + c for c in range(C) for h in range(H)], np.int64)


def _ext_weights(W, a_s, a_d, perm_rows):
    """bf16 [P, nk, 528] = [W_cm | W@A_s | W@A_d]; columns chan-major, rows
    permuted chan-major when the layer input is chan-major."""
    K = W.shape[0]
    if perm_rows:
        W = W[_PERM_CM, :]
    Wr = W.reshape(K, H, C)
    ws = np.einsum("fhc,hc->fh", Wr, a_s)
    wd = np.einsum("fhc,hc->fh", Wr, a_d)
    Wx = np.concatenate([W[:, _PERM_CM], ws, wd], axis=1).astype(np.float32)
    nk = K // P
    return np.ascontiguousarray(
        Wx.reshape(nk, P, WEXT).transpose(1, 0, 2)).astype(np_bf16)


def _build_nc(plan, repeat=1, probe=frozenset()):
    probe = frozenset(probe)
    TOTCOL = sum(r + c for r, c in plan)
    NCOLMX = max(r + c for r, c in plan)
    CMX = max(c for _, c in plan)
    nc = bacc.Bacc("TRN2", target_bir_lowering=False, debug=False,
                   num_devices=NCORES)

    x_ext = nc.dram_tensor("x0", [NSHP, F_IN], BF16, kind="ExternalInput")
    esrc_ext = nc.dram_tensor("esrc", [P, TOTCOL], I32, kind="ExternalInput")
    edst_ext = nc.dram_tensor("edst", [P, TOTCOL], F32, kind="ExternalInput")
    w1_ext = nc.dram_tensor("w1", [P, 1, WEXT], BF16, kind="ExternalInput")
    w2_ext = nc.dram_tensor("w2", [P, 4, WEXT], BF16, kind="ExternalInput")
    w3_ext = nc.dram_tensor("w3", [P, 4, WEXT], BF16, kind="ExternalInput")
    b1_ext = nc.dram_tensor("b1r", [P, HC], BF16, kind="ExternalInput")
    b2_ext = nc.dram_tensor("b2r", [P, HC], BF16, kind="ExternalInput")
    b3_ext = nc.dram_tensor("b3r", [P, C], BF16, kind="ExternalInput")
    pool_ext = nc.dram_tensor("poolidx", [P, NT], F32, kind="ExternalInput")
    invc_ext = nc.dram_tensor("invcnt", [G, 1], F32, kind="ExternalInput")
    fcw_ext = nc.dram_tensor("fcw", [C, NCLS], F32, kind="ExternalInput")
    fcb_ext = nc.dram_tensor("fcbr", [G, NCLS], F32, kind="ExternalInput")
    out_ext = nc.dram_tensor("out", [G, NCLS], F32, kind="ExternalOutput")

    rg = [list(range(NCORES))]

    with tile.TileContext(nc) as tc:
        with (
            tc.tile_pool(name="const", bufs=1) as cpool,
            tc.tile_pool(name="work", bufs=3) as wpool,
            tc.tile_pool(name="gat", bufs=3) as gpool,
            tc.tile_pool(name="ps", bufs=1, space="PSUM") as pspool,
            tc.tile_pool(name="dram", bufs=1, space="DRAM") as dpool,
        ):
            # ---- constants ----
            iota_i = cpool.tile([P, P], I32)
            nc.gpsimd.iota(iota_i[:], pattern=[[1, P]], base=0, channel_multiplier=0)
            iota_f = cpool.tile([P, P], F32)
            nc.vector.tensor_copy(iota_f[:], iota_i[:])
            identf = cpool.tile([P, P], F32)
            make_identity(nc, identf[:])
            identb = cpool.tile([P, P], BF16)
            make_identity(nc, identb[:])

            w1_s = cpool.tile([P, 1, WEXT], BF16)
            nc.sync.dma_start(out=w1_s[:], in_=w1_ext[:])
            w2_s = cpool.tile([P, 4, WEXT], BF16)
            nc.sync.dma_start(out=w2_s[:], in_=w2_ext[:])
            w3_s = cpool.tile([P, 4, WEXT], BF16)
            nc.sync.dma_start(out=w3_s[:], in_=w3_ext[:])
            b1_s = cpool.tile([P, HC], BF16)
            nc.sync.dma_start(out=b1_s[:], in_=b1_ext[:])
            b2_s = cpool.tile([P, HC], BF16)
            nc.sync.dma_start(out=b2_s[:], in_=b2_ext[:])
            b3_s = cpool.tile([P, C], BF16)
            nc.sync.dma_start(out=b3_s[:], in_=b3_ext[:])
            pool_s = cpool.tile([P, NT], F32)
            nc.sync.dma_start(out=pool_s[:], in_=pool_ext[:])
            invc_s = cpool.tile([G, 1], F32)
            nc.sync.dma_start(out=invc_s[:], in_=invc_ext[:])
            fcw_s = cpool.tile([C, NCLS], F32)
            nc.sync.dma_start(out=fcw_s[:], in_=fcw_ext[:])
            fcb_s = cpool.tile([G, NCLS], F32)
            nc.sync.dma_start(out=fcb_s[:], in_=fcb_ext[:])
            es_all = cpool.tile([P, TOTCOL], I32)
            nc.sync.dma_start(out=es_all[:], in_=esrc_ext[:])
            ed_all = cpool.tile([P, TOTCOL], F32)
            nc.sync.dma_start(out=ed_all[:], in_=edst_ext[:])
            # pad-partition mask for the last tile: rows >= PADSTART get
            # MASKNEG added to their a_src so pad-row gathers vanish pre-exp
            pidx_i = cpool.tile([P, 1], I32)
            nc.gpsimd.iota(pidx_i[:], pattern=[[0, 1]], base=0,
                           channel_multiplier=1)
            pidx_f = cpool.tile([P, 1], F32)
            nc.vector.tensor_copy(pidx_f[:], pidx_i[:])
            padmask = cpool.tile([P, 1], F32)
            nc.vector.tensor_scalar(padmask[:], pidx_f[:], PADSTART - 0.5,
                                    MASKNEG, op0=mybir.AluOpType.is_ge,
                                    op1=mybir.AluOpType.mult)
            # per-node a_dst for current/next layer (f32 adds + bf16 matmul rhs)
            adf_a = cpool.tile([P, NT * H], F32)
            adf_b = cpool.tile([P, NT * H], F32)
            adb_a = cpool.tile([P, NT * H], BF16)
            adb_b = cpool.tile([P, NT * H], BF16)

            # static one-hot tables for all chunk columns (edge data only)
            CB = [0]
            for _r, _c in plan:
                CB.append(CB[-1] + _c)
            TCH = CB[-1]
            if TCH > 0:
                oh_all = cpool.tile([P, TCH * P], BF16)
                ohT_all = cpool.tile([P, TCH * P], BF16)
                _ch0 = 0
                for _t in range(NT):
                    _R, _Ct = plan[_t]
                    if _Ct > 0:
                        _edv = ed_all[:, _ch0:_ch0 + _Ct].unsqueeze(2)\
                            .broadcast_to([P, _Ct, P])
                        _iov = iota_f[:].unsqueeze(1).broadcast_to([P, _Ct, P])
                        nc.vector.tensor_tensor(
                            out=oh_all[:, CB[_t] * P:CB[_t + 1] * P].rearrange(
                                "p (n q) -> p n q", n=_Ct, q=P),
                            in0=_edv, in1=_iov, op=mybir.AluOpType.is_equal)
                        _ohT_ps = pspool.tile([P, CCAP * P], BF16, tag="trans",
                                              bufs=2, name="ohT_ps")
                        for _cc in range(_Ct):
                            nc.tensor.transpose(
                                out=_ohT_ps[:, _cc * P:(_cc + 1) * P],
                                in_=oh_all[:, (CB[_t] + _cc) * P:
                                           (CB[_t] + _cc + 1) * P],
                                identity=identb[:])
                        nc.scalar.copy(ohT_all[:, CB[_t] * P:CB[_t + 1] * P],
                                       _ohT_ps[:, 0:_Ct * P])
                    _ch0 += _R + _Ct

            hx_local = hx_fulls = pool_in = pool_out = None

            w_tiles = (w1_s, w2_s, w3_s)
            b_tiles = (b1_s, b2_s, b3_s)
            adf_of = (adf_a, adf_b, adf_a)
            adb_of = (adb_a, adb_b, adb_a)
            split_end = {SPLIT_TILES[i + 1] - 1: i
                         for i in range(len(SPLIT_TILES) - 1)}

            # The asrc AG is emitted at the end of the producing loop; the
            # big hx AG is emitted AFTER the next alpha loop's gathers so
            # those Pool-queue gathers are not parked behind its transfer,
            # and the alpha compute overlaps the hx AG.
            def emit_asrc_ag(layer):
                if "no_ag" in probe:
                    return
                nc.gpsimd.collective_compute(
                    "AllGather", mybir.AluOpType.bypass, replica_groups=rg,
                    ins=[asrc_local[:]],
                    outs=[asrc_fulls[layer][:]],
                )

            def emit_hx_ag(layer, seg):
                if "no_ag" in probe:
                    return
                r0, r1 = SPLIT_ROWS[seg], SPLIT_ROWS[seg + 1]
                go = NCORES * r0
                nc.gpsimd.collective_compute(
                    "AllGather", mybir.AluOpType.bypass, replica_groups=rg,
                    ins=[hx_local[r0:r1, :]],
                    outs=[hx_fulls[layer][go:go + NCORES * (r1 - r0), :]],
                )

            def stage_a(xt, layer, t, dst=None, write_ad=True,
                        split_copy=False):
                """xt: SBUF bf16 [P, K] node-tile features for `layer`; emits
                [h | a_src] -> dst rows (default hx_local) and a_dst -> adf/adb."""
                K = F_IN if layer == 0 else HC
                nk = K // P
                w_s = w_tiles[layer]
                # transpose via the DMA xbar: frees PE + the Act copy and
                # breaks the per-tile PE->Act->PE chain
                xT = wpool.tile([P, HC], BF16, tag="xT", bufs=4, name="xT")
                for j in range(nk):
                    nc.sync.dma_start_transpose(
                        out=xT[:, j * P:(j + 1) * P],
                        in_=xt[:, j * P:(j + 1) * P])
                h_ps = pspool.tile([P, HC], F32, tag="big", bufs=3, name="h_ps")
                a_ps = pspool.tile([P, 2 * H], F32, tag="small", bufs=3,
                                   name="a_ps")
                for j in range(nk):
                    nc.tensor.matmul(out=h_ps[:], lhsT=xT[:, j * P:(j + 1) * P],
                                     rhs=w_s[:, j, 0:HC],
                                     start=(j == 0), stop=(j == nk - 1))
                    nc.tensor.matmul(out=a_ps[:], lhsT=xT[:, j * P:(j + 1) * P],
                                     rhs=w_s[:, j, HC:WEXT],
                                     start=(j == 0), stop=(j == nk - 1))
                hx_t = wpool.tile([P, HC], BF16, tag="hx_t", bufs=4, name="hx_t")
                if split_copy:
                    nc.scalar.copy(hx_t[:, 0:HC // 2], h_ps[:, 0:HC // 2])
                    nc.vector.tensor_copy(hx_t[:, HC // 2:HC], h_ps[:, HC // 2:HC])
                else:
                    nc.scalar.copy(hx_t[:, 0:HC], h_ps[:])
                hxa_t = wpool.tile([P, H], BF16, tag="hxa_t", bufs=4,
                                   name="hxa_t")
                if t == NT - 1:
                    # plant MASKNEG in the pad rows' a_src (pad-slot target)
                    nc.vector.tensor_tensor(
                        out=hxa_t[:], in0=a_ps[:, 0:H],
                        in1=padmask[:, 0:1].to_broadcast([P, H]),
                        op=mybir.AluOpType.add)
                else:
                    nc.vector.tensor_copy(hxa_t[:], a_ps[:, 0:H])
                nc.sync.dma_start(out=asrc_local[t * P:(t + 1) * P, :],
                                  in_=hxa_t[:])
                if write_ad:
                    adf_n = adf_of[layer]
                    adb_n = adb_of[layer]
                    nc.vector.tensor_copy(adf_n[:, t * H:(t + 1) * H],
                                          a_ps[:, H:2 * H])
                    nc.vector.tensor_copy(adb_n[:, t * H:(t + 1) * H],
                                          a_ps[:, H:2 * H])
                if dst is None:
                    dst = hx_local
                nc.sync.dma_start(out=dst[t * P:(t + 1) * P, :], in_=hx_t[:])

            # ---- layer-0 stage A (from input features) ----
            for _rep in range(repeat):
              hx_local = dpool.tile([NSHP, HC], BF16, name="hx_local")
              asrc_local = dpool.tile([NSHP, H], BF16, name="asrc_local")
              hx_fulls = [
                  dpool.tile([NFULL, HC], BF16, addr_space="Shared",
                             name=f"hx_full{i}")
                  for i in range(3)
              ]
              asrc_fulls = [
                  dpool.tile([NFULL, H], BF16, addr_space="Shared",
                             name=f"asrc_full{i}")
                  for i in range(3)
              ]
              pool_in = dpool.tile([G, C], F32, name="pool_in")
              pool_out = dpool.tile([G, C], F32, addr_space="Shared",
                                    name="pool_out")
              for t in range(NT):
                  xt_b = wpool.tile([P, F_IN], BF16, tag="xt0", name="xt0")
                  nc.sync.dma_start(out=xt_b[:], in_=x_ext[t * P:(t + 1) * P, :])
                  stage_a(xt_b, 0, t, split_copy=True)
                  if t == NT - 1:
                      emit_asrc_ag(0)

              pool_ps = None
              for layer in range(3):
                  hx_full = hx_fulls[layer]
                  asrc_full = asrc_fulls[layer]
                  b_s = b_tiles[layer]
                  adf_cur = adf_of[layer]
                  adb_cur = adb_of[layer]
                  if layer == 2:
                      pool_ps = pspool.tile([G, C], F32, tag="small", bufs=3,
                                            name="pool_ps")

                  # ==== alpha loop: stage-major tile groups =================
                  # Per group of GRP tiles each stage is emitted for every
                  # tile before the next stage, so each engine gets long
                  # same-stage instruction runs and cross-engine semaphore
                  # waits amortize over the group instead of per tile.
                  # Per-group PSUM scratch packs [adpe | den | recpe] into a
                  # single bank-sized tile (PSUM slots are bank-granular and
                  # all 8 banks are claimed by existing tags).
                  exn_all = wpool.tile([P, TOTCOL * H], BF16, tag="exnall",
                                       bufs=2, name="exn_all")
                  if "no_alpha_compute" in probe:
                      nc.vector.memset(exn_all[:], 0.25)
                  cb0 = []
                  ch0 = 0
                  for t in range(NT):
                      cb0.append(ch0)
                      ch0 += plan[t][0] + plan[t][1]
                  AD0 = 0                    # adpe region base (f32 cols)
                  DN0 = GRP * CCAP * H       # den region base
                  RC0 = DN0 + GRP * H        # recpe region base
                  for g0 in range(0, NT, GRP):
                      grp = list(range(g0, min(g0 + GRP, NT)))
                      st = {t: {} for t in grp}
                      aps = pspool.tile([P, RC0 + GRP * CCAP * H], F32,
                                        tag="small", bufs=3, name="alpha_ps")
                      # PE: a_dst for chunk edges via one-hot (needs adb only)
                      if "no_alpha_compute" not in probe:
                          for j, t in enumerate(grp):
                              R_t, C_t = plan[t]
                              if C_t == 0:
                                  continue
                              adb = adb_cur[:, t * H:(t + 1) * H]
                              ohT = ohT_all[:, CB[t] * P:CB[t + 1] * P]
                              for cc in range(C_t):
                                  nc.tensor.matmul(
                                      out=aps[:, AD0 + (j * CCAP + cc) * H:
                                              AD0 + (j * CCAP + cc + 1) * H],
                                      lhsT=ohT[:, cc * P:(cc + 1) * P],
                                      rhs=adb, start=True, stop=True)
                      # gpsimd: ONE batched a_src strip gather per group
                      gcb = cb0[grp[0]]
                      gcols = sum(plan[t][0] + plan[t][1] for t in grp)
                      ga_g = gpool.tile([P, GRP * NCOLMX * H], BF16, tag="ga",
                                        bufs=2, name="ga")
                      if "no_agather" in probe:
                          nc.vector.memset(ga_g[:, 0:gcols * H], 0.25)
                      else:
                          nc.gpsimd.indirect_dma_start(
                              out=ga_g[:, 0:gcols * H], out_offset=None,
                              in_=asrc_full[:],
                              in_offset=bass.IndirectOffsetOnAxis(
                                  ap=es_all[:, gcb:gcb + gcols], axis=0),
                          )
                      for t in grp:
                          ncol = plan[t][0] + plan[t][1]
                          st[t]["ga"] = ga_g[:, (cb0[t] - gcb) * H:
                                             (cb0[t] - gcb + ncol) * H]
                      if "no_alpha_compute" in probe:
                          continue
                      # DVE: logits (pad slots arrive as MASKNEG) + leaky relu
                      for t in grp:
                          R_t, C_t = plan[t]
                          ncol = R_t + C_t
                          j = t - g0
                          ga = st[t]["ga"]
                          adf = adf_cur[:, t * H:(t + 1) * H]
                          logit = wpool.tile([P, NCOLMX * H], F32, tag="logit",
                                             bufs=GRP + 1, name="logit")
                          lgv = logit[:, 0:ncol * H].rearrange(
                              "p (n h) -> p n h", n=ncol, h=H)
                          nc.vector.tensor_tensor(
                              out=lgv[:, C_t:ncol, :],
                              in0=ga[:, C_t * H:ncol * H].rearrange(
                                  "p (n h) -> p n h", n=R_t, h=H),
                              in1=adf[:].unsqueeze(1).broadcast_to(
                                  [P, R_t, H]),
                              op=mybir.AluOpType.add)
                          if C_t > 0:
                              nc.vector.tensor_tensor(
                                  out=lgv[:, 0:C_t, :],
                                  in0=ga[:, 0:C_t * H].rearrange(
                                      "p (n h) -> p n h", n=C_t, h=H),
                                  in1=aps[:, AD0 + j * CCAP * H:
                                          AD0 + (j * CCAP + C_t) * H]
                                      .rearrange("p (n h) -> p n h",
                                                 n=C_t, h=H),
                                  op=mybir.AluOpType.add)
                          lr = wpool.tile([P, NCOLMX * H], F32, tag="lr",
                                          bufs=GRP + 1, name="lr")
                          nc.vector.scalar_tensor_tensor(
                              out=lr[:, 0:ncol * H], in0=logit[:, 0:ncol * H],
                              scalar=NEG, in1=logit[:, 0:ncol * H],
                              op0=mybir.AluOpType.mult,
                              op1=mybir.AluOpType.max)
                          st[t]["lr"] = lr
                      # Act: exp (straight to bf16)
                      for t in grp:
                          ncol = plan[t][0] + plan[t][1]
                          ex = wpool.tile([P, NCOLMX * H], BF16, tag="ex",
                                          bufs=GRP + 1, name="ex")
                          nc.scalar.activation(
                              ex[:, 0:ncol * H], st[t]["lr"][:, 0:ncol * H],
                              mybir.ActivationFunctionType.Exp)
                          st[t]["ex"] = ex
                      # DVE: round-column denominator
                      for t in grp:
                          R_t, C_t = plan[t]
                          ncol = R_t + C_t
                          den_r = wpool.tile([P, H], F32, tag="den_r",
                                             bufs=GRP + 1, name="den_r")
                          exrv = st[t]["ex"][:, C_t * H:ncol * H].rearrange(
                              "p (n h) -> p n h", n=R_t, h=H)\
                              .transpose([0, 2, 1])
                          nc.vector.tensor_reduce(den_r[:], exrv,
                                                  axis=mybir.AxisListType.X,
                                                  op=mybir.AluOpType.add)
                          st[t]["den_r"] = den_r
                      # PE: chunk-column denominator via one-hot
                      for t in grp:
                          R_t, C_t = plan[t]
                          if C_t == 0:
                              continue
                          j = t - g0
                          oh = oh_all[:, CB[t] * P:CB[t + 1] * P]
                          for cc in range(C_t):
                              nc.tensor.matmul(
                                  out=aps[:, DN0 + j * H:DN0 + (j + 1) * H],
                                  lhsT=oh[:, cc * P:(cc + 1) * P],
                                  rhs=st[t]["ex"][:, cc * H:(cc + 1) * H],
                                  start=(cc == 0), stop=(cc == C_t - 1))
                      # DVE: total denominator + reciprocal (bf16)
                      for t in grp:
                          R_t, C_t = plan[t]
                          j = t - g0
                          rec_b = wpool.tile([P, H], BF16, tag="rec_b",
                                             bufs=GRP + 1, name="rec_b")
                          den = wpool.tile([P, H], F32, tag="den",
                                           bufs=GRP + 1, name="den")
                          if C_t > 0:
                              # EPS keeps pad nodes (no real edges) finite
                              nc.vector.scalar_tensor_tensor(
                                  out=den[:], in0=st[t]["den_r"][:],
                                  scalar=EPS,
                                  in1=aps[:, DN0 + j * H:DN0 + (j + 1) * H],
                                  op0=mybir.AluOpType.add,
                                  op1=mybir.AluOpType.add)
                          else:
                              nc.vector.tensor_scalar_add(
                                  den[:], st[t]["den_r"][:], EPS)
                          with nc.allow_low_precision(
                                  reason="1/den to bf16; alpha tol ~1e-2"):
                              nc.vector.reciprocal(rec_b[:], den[:])
                          st[t]["rec_b"] = rec_b
                      # DVE: normalized alpha, round columns
                      for t in grp:
                          R_t, C_t = plan[t]
                          ncol = R_t + C_t
                          nc.vector.tensor_tensor(
                              out=exn_all[:, (cb0[t] + C_t) * H:
                                          (cb0[t] + ncol) * H].rearrange(
                                  "p (n h) -> p n h", n=R_t, h=H),
                              in0=st[t]["ex"][:, C_t * H:ncol * H].rearrange(
                                  "p (n h) -> p n h", n=R_t, h=H),
                              in1=st[t]["rec_b"][:].unsqueeze(1)
                                  .broadcast_to([P, R_t, H]),
                              op=mybir.AluOpType.mult)
                      # PE: per-edge reciprocal for chunk columns
                      for t in grp:
                          R_t, C_t = plan[t]
                          if C_t == 0:
                              continue
                          j = t - g0
                          ohT = ohT_all[:, CB[t] * P:CB[t + 1] * P]
                          for cc in range(C_t):
                              nc.tensor.matmul(
                                  out=aps[:, RC0 + (j * CCAP + cc) * H:
                                          RC0 + (j * CCAP + cc + 1) * H],
                                  lhsT=ohT[:, cc * P:(cc + 1) * P],
                                  rhs=st[t]["rec_b"][:], start=True,
                                  stop=True)
                      # DVE: normalized alpha, chunk columns
                      for t in grp:
                          R_t, C_t = plan[t]
                          if C_t == 0:
                              continue
                          j = t - g0
                          nc.vector.tensor_tensor(
                              out=exn_all[:, cb0[t] * H:(cb0[t] + C_t) * H],
                              in0=st[t]["ex"][:, 0:C_t * H],
                              in1=aps[:, RC0 + j * CCAP * H:
                                      RC0 + (j * CCAP + C_t) * H],
                              op=mybir.AluOpType.mult)

                  # hx AG after the alpha gathers: the h-loop gathers queue
                  # behind it (they need it), alpha compute overlaps it
                  emit_hx_ag(layer, 0)

                  # ==== h loop: full-row gathers -> messages -> scatter-add ====
                  ch0 = 0
                  for t in range(NT):
                      R_t, C_t = plan[t]
                      ncol = R_t + C_t

                      gt = gpool.tile([P, NCOLMX * HC], BF16, tag="gt", bufs=4,
                                      name="gt")
                      if "no_hgather" in probe:
                          nc.vector.memset(gt[:, 0:ncol * HC], 0.25)
                      else:
                          nc.gpsimd.indirect_dma_start(
                              out=gt[:, 0:ncol * HC], out_offset=None,
                              in_=hx_full[:],
                              in_offset=bass.IndirectOffsetOnAxis(
                                  ap=es_all[:, ch0:ch0 + ncol], axis=0),
                          )
                      gtv = gt[:, 0:ncol * HC].rearrange(
                          "p (n c h) -> p n c h", n=ncol, c=C, h=H)

                      msg = gpool.tile([P, NCOLMX * HC], BF16, tag="msg",
                                       bufs=4, name="msg")
                      if "no_msg" in probe:
                          msg = gt
                      else:
                          msgv = msg[:, 0:ncol * HC].rearrange(
                              "p (n c h) -> p n c h", n=ncol, c=C, h=H)
                          exv = exn_all[:, ch0 * H:(ch0 + ncol) * H].rearrange(
                              "p (n h) -> p n h", n=ncol, h=H).unsqueeze(2)\
                              .broadcast_to([P, ncol, C, H])
                          nc.vector.tensor_tensor(out=msgv, in0=gtv, in1=exv,
                                                  op=mybir.AluOpType.mult)

                      num_ps = pspool.tile([P, HC], F32, tag="big", bufs=3,
                                           name="num_ps")
                      sc_cols = [0] if "no_scatter" in probe else list(range(ncol))
                      for cc in sc_cols:
                          lhs = (oh_all[:, (CB[t] + cc) * P:
                                        (CB[t] + cc + 1) * P] if cc < C_t
                                 else identb[:])
                          nc.tensor.matmul(out=num_ps[:], lhsT=lhs,
                                           rhs=msg[:, cc * HC:(cc + 1) * HC],
                                           start=(cc == 0),
                                           stop=(layer == 2
                                                 and cc == sc_cols[-1]))
                      if layer < 2:
                          # bias add: identity @ replicated-bias rows
                          nc.tensor.matmul(out=num_ps[:], lhsT=identb[:],
                                           rhs=b_s[:], start=False, stop=True)

                      if layer < 2:
                          xt = wpool.tile([P, HC], BF16, tag="xt", bufs=4,
                                          name="xt")
                          nc.scalar.activation(xt[:], num_ps[:],
                                               mybir.ActivationFunctionType.Relu)
                          stage_a(xt, layer + 1, t)
                          if t == NT - 1:
                              emit_asrc_ag(layer + 1)
                      else:
                          hm = wpool.tile([P, C], F32, tag="hm", name="hm")
                          nc.vector.tensor_reduce(
                              hm[:],
                              num_ps[:].rearrange("p (c h) -> p c h", c=C, h=H),
                              axis=mybir.AxisListType.X, op=mybir.AluOpType.add)
                          h3f = wpool.tile([P, C], F32, tag="h3f", name="h3f")
                          nc.vector.scalar_tensor_tensor(
                              out=h3f[:], in0=hm[:], scalar=1.0 / H,
                              in1=b3_s[:], op0=mybir.AluOpType.mult,
                              op1=mybir.AluOpType.add)
                          h3 = wpool.tile([P, C], BF16, tag="h3", name="h3")
                          nc.scalar.activation(h3[:], h3f[:],
                                               mybir.ActivationFunctionType.Relu)
                          poh = wpool.tile([P, G], BF16, tag="poh", name="poh")
                          nc.vector.tensor_tensor(
                              out=poh[:],
                              in0=pool_s[:, t:t + 1].to_broadcast([P, G]),
                              in1=iota_f[:, 0:G], op=mybir.AluOpType.is_equal)
                          nc.tensor.matmul(out=pool_ps[:], lhsT=poh[:],
                                           rhs=h3[:], start=(t == 0),
                                           stop=(t == NT - 1))
                      ch0 += ncol

              # ---- pool AllReduce + FC + log_softmax ----
              psb = wpool.tile([G, C], F32)
              nc.vector.tensor_copy(psb[:], pool_ps[:])
              nc.sync.dma_start(out=pool_in[:], in_=psb[:])
              nc.gpsimd.collective_compute(
                  "AllReduce", mybir.AluOpType.add, replica_groups=rg,
                  ins=[pool_in[:]], outs=[pool_out[:]],
              )
              pld = wpool.tile([G, C], F32)
              nc.sync.dma_start(out=pld[:], in_=pool_out[:])
              nc.vector.tensor_scalar_mul(pld[:], pld[:], invc_s[:, 0:1])
              pT_ps = pspool.tile([G, C], F32, tag="small", bufs=3, name="pT_ps")
              nc.tensor.transpose(out=pT_ps[:], in_=pld[:], identity=identf[0:G, 0:C])
              pT = wpool.tile([C, G], F32)
              nc.scalar.copy(pT[:], pT_ps[:])
              z_ps = pspool.tile([G, NCLS], F32, tag="small", bufs=3, name="z_ps")
              nc.tensor.matmul(out=z_ps[:], lhsT=pT[:], rhs=fcw_s[:],
                               start=True, stop=True)
              z = wpool.tile([G, NCLS], F32)
              nc.vector.tensor_add(z[:], z_ps[:], fcb_s[:])
              zm = wpool.tile([G, 1], F32)
              nc.vector.tensor_reduce(zm[:], z[:], axis=mybir.AxisListType.X,
                                      op=mybir.AluOpType.max)
              zs = wpool.tile([G, NCLS], F32)
              nc.vector.tensor_scalar_sub(zs[:], z[:], zm[:, 0:1])
              ze = wpool.tile([G, NCLS], F32)
              nc.scalar.activation(ze[:], zs[:], mybir.ActivationFunctionType.Exp)
              zsum = wpool.tile([G, 1], F32)
              nc.vector.tensor_reduce(zsum[:], ze[:], axis=mybir.AxisListType.X,
                                      op=mybir.AluOpType.add)
              zl = wpool.tile([G, 1], F32)
              nc.scalar.activation(zl[:], zsum[:], mybir.ActivationFunctionType.Ln)
              zo = wpool.tile([G, NCLS], F32)
              nc.vector.tensor_scalar_sub(zo[:], zs[:], zl[:, 0:1])
              nc.sync.dma_start(out=out_ext[:], in_=zo[:])

    nc.compile()
    return nc


def _prepare_inputs(x, edge_index, batch, W1, a1s, a1d, b1, W2, a2s, a2d, b2,
                    W3, a3s, a3d, b3, fcw, fcb):
    plan, esrcT, edstT = _preprocess_edges(
        np.asarray(edge_index, np.int64))
    x = np.asarray(x, np.float32)
    batch = np.asarray(batch, np.int64)

    shared = {
        "w1": _ext_weights(np.asarray(W1, np.float32), np.asarray(a1s, np.float32),
                           np.asarray(a1d, np.float32), perm_rows=False),
        "w2": _ext_weights(np.asarray(W2, np.float32), np.asarray(a2s, np.float32),
                           np.asarray(a2d, np.float32), perm_rows=True),
        "w3": _ext_weights(np.asarray(W3, np.float32), np.asarray(a3s, np.float32),
                           np.asarray(a3d, np.float32), perm_rows=True),
        "b1r": np.tile(np.asarray(b1, np.float32)[_PERM_CM][None, :],
                       (P, 1)).astype(np_bf16),
        "b2r": np.tile(np.asarray(b2, np.float32)[_PERM_CM][None, :],
                       (P, 1)).astype(np_bf16),
        "b3r": np.tile(np.asarray(b3, np.float32)[None, :], (P, 1)).astype(np_bf16),
        "fcw": np.asarray(fcw, np.float32),
        "fcbr": np.tile(np.asarray(fcb, np.float32)[None, :], (G, 1)),
        "invcnt": (1.0 / np.maximum(
            np.bincount(batch, minlength=G), 1.0)).astype(np.float32)[:, None],
    }

    in_maps = []
    for k in range(NCORES):
        xk = np.zeros((NSHP, F_IN), np_bf16)
        xk[:NSH] = x[k * NSH:(k + 1) * NSH].astype(np_bf16)
        pidx = np.full((NSHP,), PAD, np.float32)
        pidx[:NSH] = batch[k * NSH:(k + 1) * NSH]
        poolidx = np.ascontiguousarray(pidx.reshape(NT, P).T)  # [P, NT]
        in_maps.append({
            "x0": xk,
            "esrc": np.ascontiguousarray(esrcT[k]),
            "edst": np.ascontiguousarray(edstT[k]),
            "poolidx": poolidx,
            **shared,
        })
    return plan, in_maps


_CACHE = {}


def _get_nc(plan, repeat=1, probe=frozenset()):
    key = (tuple(plan), repeat, tuple(sorted(probe)))
    if key not in _CACHE:
        _CACHE[key] = _build_nc(plan, repeat, probe)
    return _CACHE[key]


def kernel(x, edge_index, batch, W1, a1s, a1d, b1, W2, a2s, a2d, b2,
           W3, a3s, a3d, b3, fcw, fcb, _trace=False, _results=None):
    plan, in_maps = _prepare_inputs(x, edge_index, batch, W1, a1s, a1d, b1,
                                    W2, a2s, a2d, b2, W3, a3s, a3d, b3, fcw, fcb)
    nc = _get_nc(plan)
    res = run_bass_kernel_spmd(nc, in_maps, core_ids=list(range(NCORES)),
                               trace=_trace)
    if _results is not None:
        _results.append(res)
    return res.results[0]["out"]



# revision 46
# speedup vs baseline: 1.6728x; 1.6728x over previous
"""3-layer GAT (8 heads x 64 ch) + global mean pool + FC + log_softmax on 8 Trainium2 cores.

v2 redesign of the edge phase around batched per-tile operations (vs the
per-column baseline):
- Nodes (and their incoming edges) partitioned across 8 cores; small GAT
  weights replicated. Per layer each core computes [h | a_src] rows (bf16,
  channel-major) for its node shard and AllGathers them into a replicated
  DRAM table; the graph-level mean pool is the only AllReduce.
- Per 128-dst-node tile: ONE wide indirect DMA gathers all edge source rows
  (multi-column offset AP amortizes the ~1us SWDGE descriptor-generation
  cost ~10x), and the attention math runs as a handful of wide strided-AP
  ops. Channel-major rows put the 8 heads innermost so the alpha broadcast
  views keep packed innermost strides (DVE 2x bf16 mode) for the big
  h*alpha multiply.
- alpha is pre-normalized (ex * 1/den) before the message multiply; for
  one-hot overflow chunks the per-edge denominator is fetched through the
  same one-hot matmul used for a_dst. This removes the per-tile
  divide-from-PSUM and lets the layer bias fold into the PSUM accumulation
  as an identity-matmul of replicated bias rows, so the finalize is a
  single Relu activation straight out of PSUM.
- Engine placement: Pool runs the gathers + SBUF-only logit adds; DVE the
  one-hot builds, denominators and the msg multiply; Act exp/copies/relu;
  PE transposes, scatter-add and the fused next-layer x@W (stage A), which
  hides under the gather-DMA phase. bf16 matmuls run at 1 cycle/row (fp32
  is 4).
- `repeat=` builds the body K times (fresh DRAM tables per rep) purely for
  timing: (T(K)-T(1))/(K-1) cancels the ~80ms axon RPC floor.
"""

import numpy as np
from ml_dtypes import bfloat16 as np_bf16

import concourse.bass as bass
import concourse.mybir as mybir
import concourse.tile as tile
from concourse import bacc
from concourse.bass_utils import run_bass_kernel_spmd
from concourse.masks import make_identity

# problem constants (hardcoded per contract)
N, E, F_IN, H, C, G, NCLS = 50000, 400000, 128, 8, 64, 64, 10
HC = H * C  # 512
NEG = 0.2
EPS = 1e-16

NCORES = 8
P = 128
NSH = N // NCORES          # 6250 nodes per core
NT = (NSH + P - 1) // P    # 49 dst tiles per core
NSHP = NT * P              # 6272 padded rows per core
NFULL = NCORES * NSHP      # 50176 rows in the gathered table
ROWW = HC + H              # 520: h (chan-major) | a_src
WEXT = HC + 2 * H          # 528: W | Wa_src | Wa_dst
PAD = 999.0                # one-hot miss marker for padded overflow slots
MASKNEG = -1.0e30          # a_src value planted in the pad row (=> exp -> 0)
CCAP = 4                   # max one-hot chunk columns per tile (PSUM bank)
NCAP = 16                  # max total columns per tile (SBUF budget)
# hx AllGather split boundaries (tile granularity). A Shared DRAM tensor
# may only be written by one instruction, so the hx AG is a single segment.
SPLIT_TILES = (0, 49)
SPLIT_ROWS = tuple(t * P for t in SPLIT_TILES)
# pad slots gather the pad row (core 0, local row NSHP-1) whose a_src holds
# MASKNEG, so padded round edges contribute exp(-inf)=0 without a mask op.
PADGID_H = NCORES * SPLIT_ROWS[-2] + (NSHP - 1 - SPLIT_ROWS[-2])
PADGID_A = NSHP - 1
PADSTART = NSH - (NT - 1) * P  # first pad partition in the last tile
GRP = 7                    # alpha-loop stage-major group size (tiles)

F32 = mybir.dt.float32
BF16 = mybir.dt.bfloat16
I32 = mybir.dt.int32


def _preprocess_edges(edge_index):
    """Assign edges (incl. self loops) to the dst-owning core; build per-tile
    round columns (node's r-th edge in its own partition) plus overflow
    one-hot chunk columns.

    Returns (plan, esrcT, edstT):
      plan: list of (R_t, C_t) per tile — shared by all cores.
      esrcT[k] int32 [P, TOTCOL]: hx gather row ids, seg-major per
        SPLIT_ROWS (pad -> PADGID_H, whose a_src holds MASKNEG).
      edstT[k] f32 [P, TOTCOL]: chunk cols -> dst slot or PAD (round cols
        unused by the kernel)."""
    src = np.concatenate([edge_index[0], np.arange(N, dtype=np.int64)])
    dst = np.concatenate([edge_index[1], np.arange(N, dtype=np.int64)])
    core = dst // NSH
    dloc = dst - core * NSH
    tile_of = dloc // P
    slot = dloc - tile_of * P
    # hx gather table layout: SPLITS segments, each rank-major over its rows
    sk = src // NSH
    sr = src % NSH
    split_rows = np.asarray(SPLIT_ROWS, np.int64)
    seg = np.searchsorted(split_rows[1:], sr, side="right")
    r0 = split_rows[seg]
    r1 = split_rows[seg + 1]
    gid = NCORES * r0 + sk * (r1 - r0) + (sr - r0)

    deg = np.zeros((NCORES, NT, P), np.int64)
    np.add.at(deg, (core, tile_of, slot), 1)
    maxdeg_t = deg.max(axis=(0, 2))  # [NT]

    # choose the round cap per tile: chunk columns carry extra one-hot work
    R_ts = np.zeros(NT, np.int64)
    C_ts = np.zeros(NT, np.int64)
    for t in range(NT):
        best = None
        for rcap in range(1, int(maxdeg_t[t]) + 1):
            r = min(int(maxdeg_t[t]), rcap)
            ovf = np.maximum(deg[:, t, :] - rcap, 0).sum(axis=1)
            c = int(np.ceil(ovf / P).max())
            if c > CCAP or r + c > NCAP:
                continue
            cost = r + 1.35 * c
            if best is None or cost < best[0]:
                best = (cost, r, c)
        assert best is not None, f"no feasible plan for tile {t}"
        _, R_ts[t], C_ts[t] = best
    plan = [(int(R_ts[t]), int(C_ts[t])) for t in range(NT)]
    colbase = np.zeros(NT, np.int64)
    colbase[1:] = np.cumsum(R_ts + C_ts)[:-1]
    TOTCOL = int((R_ts + C_ts).sum())

    esrcT = np.full((NCORES, P, TOTCOL), PADGID_H, np.int32)
    edstT = np.empty((NCORES, P, TOTCOL), np.float32)
    for k in range(NCORES):
        # default fill: chunks PAD (rounds unused)
        for t in range(NT):
            b = colbase[t]
            edstT[k, :, b:b + C_ts[t]] = PAD
            edstT[k, :, b + C_ts[t]:b + C_ts[t] + R_ts[t]] = 0.0
        m = core == k
        t_k, s_k, g_k = tile_of[m], slot[m], gid[m]
        order = np.argsort(t_k * P + s_k, kind="stable")
        t_k, s_k, g_k = t_k[order], s_k[order], g_k[order]
        node = t_k * P + s_k
        start = np.zeros(NT * P + 1, np.int64)
        np.add.at(start[1:], node, 1)
        start = np.cumsum(start)
        j = np.arange(len(node)) - start[node]  # rank within node
        rmax = R_ts[t_k]
        isr = j < rmax
        # round entries (after the C_t chunk columns)
        rcol = colbase[t_k[isr]] + C_ts[t_k[isr]] + j[isr]
        esrcT[k, s_k[isr], rcol] = g_k[isr].astype(np.int32)
        # overflow entries: sequential position within each tile
        to, so, go = t_k[~isr], s_k[~isr], g_k[~isr]
        oorder = np.argsort(to, kind="stable")
        to, so, go = to[oorder], so[oorder], go[oorder]
        ostart = np.zeros(NT + 1, np.int64)
        np.add.at(ostart[1:], to, 1)
        ostart = np.cumsum(ostart)
        q = np.arange(len(to)) - ostart[to]
        col = colbase[to] + q // P
        row = q % P
        esrcT[k, row, col] = go.astype(np.int32)
        edstT[k, row, col] = so.astype(np.float32)
    return plan, esrcT, edstT


# channel-major permutation: cm index (c*H + h) <- standard (h*C + c)
_PERM_CM = np.array([h * C + c for c in range(C) for h in range(H)], np.int64)


def _ext_weights(W, a_s, a_d, perm_rows):
    """bf16 [P, nk, 528] = [W_cm | W@A_s | W@A_d]; columns chan-major, rows
    permuted chan-major when the layer input is chan-major."""
    K = W.shape[0]
    if perm_rows:
        W = W[_PERM_CM, :]
    Wr = W.reshape(K, H, C)
    ws = np.einsum("fhc,hc->fh", Wr, a_s)
    wd = np.einsum("fhc,hc->fh", Wr, a_d)
    Wx = np.concatenate([W[:, _PERM_CM], ws, wd], axis=1).astype(np.float32)
    nk = K // P
    return np.ascontiguousarray(
        Wx.reshape(nk, P, WEXT).transpose(1, 0, 2)).astype(np_bf16)


def _build_nc(plan, repeat=1, probe=frozenset()):
    probe = frozenset(probe)
    TOTCOL = sum(r + c for r, c in plan)
    NCOLMX = max(r + c for r, c in plan)
    CMX = max(c for _, c in plan)
    nc = bacc.Bacc("TRN2", target_bir_lowering=False, debug=False,
                   num_devices=NCORES)

    x_ext = nc.dram_tensor("x0", [NSHP, F_IN], BF16, kind="ExternalInput")
    esrc_ext = nc.dram_tensor("esrc", [P, TOTCOL], I32, kind="ExternalInput")
    edst_ext = nc.dram_tensor("edst", [P, TOTCOL], F32, kind="ExternalInput")
    w1_ext = nc.dram_tensor("w1", [P, 1, WEXT], BF16, kind="ExternalInput")
    w2_ext = nc.dram_tensor("w2", [P, 4, WEXT], BF16, kind="ExternalInput")
    w3_ext = nc.dram_tensor("w3", [P, 4, WEXT], BF16, kind="ExternalInput")
    b1_ext = nc.dram_tensor("b1r", [P, HC], BF16, kind="ExternalInput")
    b2_ext = nc.dram_tensor("b2r", [P, HC], BF16, kind="ExternalInput")
    b3_ext = nc.dram_tensor("b3r", [P, C], BF16, kind="ExternalInput")
    pool_ext = nc.dram_tensor("poolidx", [P, NT], F32, kind="ExternalInput")
    invc_ext = nc.dram_tensor("invcnt", [G, 1], F32, kind="ExternalInput")
    fcw_ext = nc.dram_tensor("fcw", [C, NCLS], F32, kind="ExternalInput")
    fcb_ext = nc.dram_tensor("fcbr", [G, NCLS], F32, kind="ExternalInput")
    out_ext = nc.dram_tensor("out", [G, NCLS], F32, kind="ExternalOutput")

    rg = [list(range(NCORES))]

    with tile.TileContext(nc) as tc:
        with (
            tc.tile_pool(name="const", bufs=1) as cpool,
            tc.tile_pool(name="work", bufs=3) as wpool,
            tc.tile_pool(name="gat", bufs=3) as gpool,
            tc.tile_pool(name="ps", bufs=1, space="PSUM") as pspool,
            tc.tile_pool(name="dram", bufs=1, space="DRAM") as dpool,
        ):
            # ---- constants ----
            iota_i = cpool.tile([P, P], I32)
            nc.gpsimd.iota(iota_i[:], pattern=[[1, P]], base=0, channel_multiplier=0)
            iota_f = cpool.tile([P, P], F32)
            nc.vector.tensor_copy(iota_f[:], iota_i[:])
            identf = cpool.tile([P, P], F32)
            make_identity(nc, identf[:])
            identb = cpool.tile([P, P], BF16)
            make_identity(nc, identb[:])

            w1_s = cpool.tile([P, 1, WEXT], BF16)
            nc.sync.dma_start(out=w1_s[:], in_=w1_ext[:])
            w2_s = cpool.tile([P, 4, WEXT], BF16)
            nc.sync.dma_start(out=w2_s[:], in_=w2_ext[:])
            w3_s = cpool.tile([P, 4, WEXT], BF16)
            nc.sync.dma_start(out=w3_s[:], in_=w3_ext[:])
            b1_s = cpool.tile([P, HC], BF16)
            nc.sync.dma_start(out=b1_s[:], in_=b1_ext[:])
            b2_s = cpool.tile([P, HC], BF16)
            nc.sync.dma_start(out=b2_s[:], in_=b2_ext[:])
            b3_s = cpool.tile([P, C], BF16)
            nc.sync.dma_start(out=b3_s[:], in_=b3_ext[:])
            pool_s = cpool.tile([P, NT], F32)
            nc.sync.dma_start(out=pool_s[:], in_=pool_ext[:])
            invc_s = cpool.tile([G, 1], F32)
            nc.sync.dma_start(out=invc_s[:], in_=invc_ext[:])
            fcw_s = cpool.tile([C, NCLS], F32)
            nc.sync.dma_start(out=fcw_s[:], in_=fcw_ext[:])
            fcb_s = cpool.tile([G, NCLS], F32)
            nc.sync.dma_start(out=fcb_s[:], in_=fcb_ext[:])
            es_all = cpool.tile([P, TOTCOL], I32)
            nc.sync.dma_start(out=es_all[:], in_=esrc_ext[:])
            ed_all = cpool.tile([P, TOTCOL], F32)
            nc.sync.dma_start(out=ed_all[:], in_=edst_ext[:])
            # pad-partition mask for the last tile: rows >= PADSTART get
            # MASKNEG added to their a_src so pad-row gathers vanish pre-exp
            pidx_i = cpool.tile([P, 1], I32)
            nc.gpsimd.iota(pidx_i[:], pattern=[[0, 1]], base=0,
                           channel_multiplier=1)
            pidx_f = cpool.tile([P, 1], F32)
            nc.vector.tensor_copy(pidx_f[:], pidx_i[:])
            padmask = cpool.tile([P, 1], F32)
            nc.vector.tensor_scalar(padmask[:], pidx_f[:], PADSTART - 0.5,
                                    MASKNEG, op0=mybir.AluOpType.is_ge,
                                    op1=mybir.AluOpType.mult)
            # per-node a_dst for current/next layer (f32 adds + bf16 matmul rhs)
            adf_a = cpool.tile([P, NT * H], F32)
            adf_b = cpool.tile([P, NT * H], F32)
            adb_a = cpool.tile([P, NT * H], BF16)
            adb_b = cpool.tile([P, NT * H], BF16)

            # static one-hot tables for all chunk columns (edge data only)
            CB = [0]
            for _r, _c in plan:
                CB.append(CB[-1] + _c)
            TCH = CB[-1]
            if TCH > 0:
                oh_all = cpool.tile([P, TCH * P], BF16)
                ohT_all = cpool.tile([P, TCH * P], BF16)
                _ch0 = 0
                for _t in range(NT):
                    _R, _Ct = plan[_t]
                    if _Ct > 0:
                        _edv = ed_all[:, _ch0:_ch0 + _Ct].unsqueeze(2)\
                            .broadcast_to([P, _Ct, P])
                        _iov = iota_f[:].unsqueeze(1).broadcast_to([P, _Ct, P])
                        nc.vector.tensor_tensor(
                            out=oh_all[:, CB[_t] * P:CB[_t + 1] * P].rearrange(
                                "p (n q) -> p n q", n=_Ct, q=P),
                            in0=_edv, in1=_iov, op=mybir.AluOpType.is_equal)
                        _ohT_ps = pspool.tile([P, CCAP * P], BF16, tag="trans",
                                              bufs=2, name="ohT_ps")
                        for _cc in range(_Ct):
                            nc.tensor.transpose(
                                out=_ohT_ps[:, _cc * P:(_cc + 1) * P],
                                in_=oh_all[:, (CB[_t] + _cc) * P:
                                           (CB[_t] + _cc + 1) * P],
                                identity=identb[:])
                        nc.scalar.copy(ohT_all[:, CB[_t] * P:CB[_t + 1] * P],
                                       _ohT_ps[:, 0:_Ct * P])
                    _ch0 += _R + _Ct

            hx_local = hx_fulls = pool_in = pool_out = None

            w_tiles = (w1_s, w2_s, w3_s)
            b_tiles = (b1_s, b2_s, b3_s)
            adf_of = (adf_a, adf_b, adf_a)
            adb_of = (adb_a, adb_b, adb_a)
            split_end = {SPLIT_TILES[i + 1] - 1: i
                         for i in range(len(SPLIT_TILES) - 1)}

            # The asrc AG is emitted at the end of the producing loop; the
            # big hx AG is emitted AFTER the next alpha loop's gathers so
            # those Pool-queue gathers are not parked behind its transfer,
            # and the alpha compute overlaps the hx AG.
            def emit_asrc_ag(layer):
                if "no_ag" in probe:
                    return
                nc.gpsimd.collective_compute(
                    "AllGather", mybir.AluOpType.bypass, replica_groups=rg,
                    ins=[asrc_local[:]],
                    outs=[asrc_fulls[layer][:]],
                )

            def emit_hx_ag(layer, seg):
                if "no_ag" in probe:
                    return
                r0, r1 = SPLIT_ROWS[seg], SPLIT_ROWS[seg + 1]
                go = NCORES * r0
                nc.gpsimd.collective_compute(
                    "AllGather", mybir.AluOpType.bypass, replica_groups=rg,
                    ins=[hx_local[r0:r1, :]],
                    outs=[hx_fulls[layer][go:go + NCORES * (r1 - r0), :]],
                )

            def stage_a(xt, layer, t, dst=None, write_ad=True,
                        split_copy=False):
                """xt: SBUF bf16 [P, K] node-tile features for `layer`; emits
                [h | a_src] -> dst rows (default hx_local) and a_dst -> adf/adb."""
                K = F_IN if layer == 0 else HC
                nk = K // P
                w_s = w_tiles[layer]
                xT_ps = pspool.tile([P, HC], BF16, tag="trans", bufs=2,
                                    name="xT_ps")
                for j in range(nk):
                    nc.tensor.transpose(out=xT_ps[:, j * P:(j + 1) * P],
                                        in_=xt[:, j * P:(j + 1) * P],
                                        identity=identb[:])
                xT = wpool.tile([P, HC], BF16, tag="xT", bufs=4, name="xT")
                nc.scalar.copy(xT[:, 0:K], xT_ps[:, 0:K])
                h_ps = pspool.tile([P, HC], F32, tag="big", bufs=3, name="h_ps")
                a_ps = pspool.tile([P, 2 * H], F32, tag="small", bufs=3,
                                   name="a_ps")
                for j in range(nk):
                    nc.tensor.matmul(out=h_ps[:], lhsT=xT[:, j * P:(j + 1) * P],
                                     rhs=w_s[:, j, 0:HC],
                                     start=(j == 0), stop=(j == nk - 1))
                    nc.tensor.matmul(out=a_ps[:], lhsT=xT[:, j * P:(j + 1) * P],
                                     rhs=w_s[:, j, HC:WEXT],
                                     start=(j == 0), stop=(j == nk - 1))
                hx_t = wpool.tile([P, HC], BF16, tag="hx_t", bufs=4, name="hx_t")
                if split_copy:
                    nc.scalar.copy(hx_t[:, 0:HC // 2], h_ps[:, 0:HC // 2])
                    nc.vector.tensor_copy(hx_t[:, HC // 2:HC], h_ps[:, HC // 2:HC])
                else:
                    nc.scalar.copy(hx_t[:, 0:HC], h_ps[:])
                hxa_t = wpool.tile([P, H], BF16, tag="hxa_t", bufs=4,
                                   name="hxa_t")
                if t == NT - 1:
                    # plant MASKNEG in the pad rows' a_src (pad-slot target)
                    nc.vector.tensor_tensor(
                        out=hxa_t[:], in0=a_ps[:, 0:H],
                        in1=padmask[:, 0:1].to_broadcast([P, H]),
                        op=mybir.AluOpType.add)
                else:
                    nc.vector.tensor_copy(hxa_t[:], a_ps[:, 0:H])
                nc.sync.dma_start(out=asrc_local[t * P:(t + 1) * P, :],
                                  in_=hxa_t[:])
                if write_ad:
                    adf_n = adf_of[layer]
                    adb_n = adb_of[layer]
                    nc.vector.tensor_copy(adf_n[:, t * H:(t + 1) * H],
                                          a_ps[:, H:2 * H])
                    nc.vector.tensor_copy(adb_n[:, t * H:(t + 1) * H],
                                          a_ps[:, H:2 * H])
                if dst is None:
                    dst = hx_local
                nc.sync.dma_start(out=dst[t * P:(t + 1) * P, :], in_=hx_t[:])

            # ---- layer-0 stage A (from input features) ----
            for _rep in range(repeat):
              hx_local = dpool.tile([NSHP, HC], BF16, name="hx_local")
              asrc_local = dpool.tile([NSHP, H], BF16, name="asrc_local")
              hx_fulls = [
                  dpool.tile([NFULL, HC], BF16, addr_space="Shared",
                             name=f"hx_full{i}")
                  for i in range(3)
              ]
              asrc_fulls = [
                  dpool.tile([NFULL, H], BF16, addr_space="Shared",
                             name=f"asrc_full{i}")
                  for i in range(3)
              ]
              pool_in = dpool.tile([G, C], F32, name="pool_in")
              pool_out = dpool.tile([G, C], F32, addr_space="Shared",
                                    name="pool_out")
              for t in range(NT):
                  xt_b = wpool.tile([P, F_IN], BF16, tag="xt0", name="xt0")
                  nc.sync.dma_start(out=xt_b[:], in_=x_ext[t * P:(t + 1) * P, :])
                  stage_a(xt_b, 0, t, split_copy=True)
                  if t == NT - 1:
                      emit_asrc_ag(0)

              pool_ps = None
              for layer in range(3):
                  hx_full = hx_fulls[layer]
                  asrc_full = asrc_fulls[layer]
                  b_s = b_tiles[layer]
                  adf_cur = adf_of[layer]
                  adb_cur = adb_of[layer]
                  if layer == 2:
                      pool_ps = pspool.tile([G, C], F32, tag="small", bufs=3,
                                            name="pool_ps")

                  # ==== alpha loop: stage-major tile groups =================
                  # Per group of GRP tiles each stage is emitted for every
                  # tile before the next stage, so each engine gets long
                  # same-stage instruction runs and cross-engine semaphore
                  # waits amortize over the group instead of per tile.
                  # Per-group PSUM scratch packs [adpe | den | recpe] into a
                  # single bank-sized tile (PSUM slots are bank-granular and
                  # all 8 banks are claimed by existing tags).
                  exn_all = wpool.tile([P, TOTCOL * H], BF16, tag="exnall",
                                       bufs=2, name="exn_all")
                  if "no_alpha_compute" in probe:
                      nc.vector.memset(exn_all[:], 0.25)
                  cb0 = []
                  ch0 = 0
                  for t in range(NT):
                      cb0.append(ch0)
                      ch0 += plan[t][0] + plan[t][1]
                  AD0 = 0                    # adpe region base (f32 cols)
                  DN0 = GRP * CCAP * H       # den region base
                  RC0 = DN0 + GRP * H        # recpe region base
                  for g0 in range(0, NT, GRP):
                      grp = list(range(g0, min(g0 + GRP, NT)))
                      st = {t: {} for t in grp}
                      aps = pspool.tile([P, RC0 + GRP * CCAP * H], F32,
                                        tag="small", bufs=3, name="alpha_ps")
                      # PE: a_dst for chunk edges via one-hot (needs adb only)
                      if "no_alpha_compute" not in probe:
                          for j, t in enumerate(grp):
                              R_t, C_t = plan[t]
                              if C_t == 0:
                                  continue
                              adb = adb_cur[:, t * H:(t + 1) * H]
                              ohT = ohT_all[:, CB[t] * P:CB[t + 1] * P]
                              for cc in range(C_t):
                                  nc.tensor.matmul(
                                      out=aps[:, AD0 + (j * CCAP + cc) * H:
                                              AD0 + (j * CCAP + cc + 1) * H],
                                      lhsT=ohT[:, cc * P:(cc + 1) * P],
                                      rhs=adb, start=True, stop=True)
                      # gpsimd: ONE batched a_src strip gather per group
                      gcb = cb0[grp[0]]
                      gcols = sum(plan[t][0] + plan[t][1] for t in grp)
                      ga_g = gpool.tile([P, GRP * NCOLMX * H], BF16, tag="ga",
                                        bufs=2, name="ga")
                      if "no_agather" in probe:
                          nc.vector.memset(ga_g[:, 0:gcols * H], 0.25)
                      else:
                          nc.gpsimd.indirect_dma_start(
                              out=ga_g[:, 0:gcols * H], out_offset=None,
                              in_=asrc_full[:],
                              in_offset=bass.IndirectOffsetOnAxis(
                                  ap=es_all[:, gcb:gcb + gcols], axis=0),
                          )
                      for t in grp:
                          ncol = plan[t][0] + plan[t][1]
                          st[t]["ga"] = ga_g[:, (cb0[t] - gcb) * H:
                                             (cb0[t] - gcb + ncol) * H]
                      if "no_alpha_compute" in probe:
                          continue
                      # DVE: logits (pad slots arrive as MASKNEG) + leaky relu
                      for t in grp:
                          R_t, C_t = plan[t]
                          ncol = R_t + C_t
                          j = t - g0
                          ga = st[t]["ga"]
                          adf = adf_cur[:, t * H:(t + 1) * H]
                          logit = wpool.tile([P, NCOLMX * H], F32, tag="logit",
                                             bufs=GRP + 1, name="logit")
                          lgv = logit[:, 0:ncol * H].rearrange(
                              "p (n h) -> p n h", n=ncol, h=H)
                          nc.vector.tensor_tensor(
                              out=lgv[:, C_t:ncol, :],
                              in0=ga[:, C_t * H:ncol * H].rearrange(
                                  "p (n h) -> p n h", n=R_t, h=H),
                              in1=adf[:].unsqueeze(1).broadcast_to(
                                  [P, R_t, H]),
                              op=mybir.AluOpType.add)
                          if C_t > 0:
                              nc.vector.tensor_tensor(
                                  out=lgv[:, 0:C_t, :],
                                  in0=ga[:, 0:C_t * H].rearrange(
                                      "p (n h) -> p n h", n=C_t, h=H),
                                  in1=aps[:, AD0 + j * CCAP * H:
                                          AD0 + (j * CCAP + C_t) * H]
                                      .rearrange("p (n h) -> p n h",
                                                 n=C_t, h=H),
                                  op=mybir.AluOpType.add)
                          lr = wpool.tile([P, NCOLMX * H], F32, tag="lr",
                                          bufs=GRP + 1, name="lr")
                          nc.vector.scalar_tensor_tensor(
                              out=lr[:, 0:ncol * H], in0=logit[:, 0:ncol * H],
                              scalar=NEG, in1=logit[:, 0:ncol * H],
                              op0=mybir.AluOpType.mult,
                              op1=mybir.AluOpType.max)
                          st[t]["lr"] = lr
                      # Act: exp (straight to bf16)
                      for t in grp:
                          ncol = plan[t][0] + plan[t][1]
                          ex = wpool.tile([P, NCOLMX * H], BF16, tag="ex",
                                          bufs=GRP + 1, name="ex")
                          nc.scalar.activation(
                              ex[:, 0:ncol * H], st[t]["lr"][:, 0:ncol * H],
                              mybir.ActivationFunctionType.Exp)
                          st[t]["ex"] = ex
                      # DVE: round-column denominator
                      for t in grp:
                          R_t, C_t = plan[t]
                          ncol = R_t + C_t
                          den_r = wpool.tile([P, H], F32, tag="den_r",
                                             bufs=GRP + 1, name="den_r")
                          exrv = st[t]["ex"][:, C_t * H:ncol * H].rearrange(
                              "p (n h) -> p n h", n=R_t, h=H)\
                              .transpose([0, 2, 1])
                          nc.vector.tensor_reduce(den_r[:], exrv,
                                                  axis=mybir.AxisListType.X,
                                                  op=mybir.AluOpType.add)
                          st[t]["den_r"] = den_r
                      # PE: chunk-column denominator via one-hot
                      for t in grp:
                          R_t, C_t = plan[t]
                          if C_t == 0:
                              continue
                          j = t - g0
                          oh = oh_all[:, CB[t] * P:CB[t + 1] * P]
                          for cc in range(C_t):
                              nc.tensor.matmul(
                                  out=aps[:, DN0 + j * H:DN0 + (j + 1) * H],
                                  lhsT=oh[:, cc * P:(cc + 1) * P],
                                  rhs=st[t]["ex"][:, cc * H:(cc + 1) * H],
                                  start=(cc == 0), stop=(cc == C_t - 1))
                      # DVE: total denominator + reciprocal (bf16)
                      for t in grp:
                          R_t, C_t = plan[t]
                          j = t - g0
                          rec_b = wpool.tile([P, H], BF16, tag="rec_b",
                                             bufs=GRP + 1, name="rec_b")
                          den = wpool.tile([P, H], F32, tag="den",
                                           bufs=GRP + 1, name="den")
                          if C_t > 0:
                              # EPS keeps pad nodes (no real edges) finite
                              nc.vector.scalar_tensor_tensor(
                                  out=den[:], in0=st[t]["den_r"][:],
                                  scalar=EPS,
                                  in1=aps[:, DN0 + j * H:DN0 + (j + 1) * H],
                                  op0=mybir.AluOpType.add,
                                  op1=mybir.AluOpType.add)
                          else:
                              nc.vector.tensor_scalar_add(
                                  den[:], st[t]["den_r"][:], EPS)
                          with nc.allow_low_precision(
                                  reason="1/den to bf16; alpha tol ~1e-2"):
                              nc.vector.reciprocal(rec_b[:], den[:])
                          st[t]["rec_b"] = rec_b
                      # DVE: normalized alpha, round columns
                      for t in grp:
                          R_t, C_t = plan[t]
                          ncol = R_t + C_t
                          nc.vector.tensor_tensor(
                              out=exn_all[:, (cb0[t] + C_t) * H:
                                          (cb0[t] + ncol) * H].rearrange(
                                  "p (n h) -> p n h", n=R_t, h=H),
                              in0=st[t]["ex"][:, C_t * H:ncol * H].rearrange(
                                  "p (n h) -> p n h", n=R_t, h=H),
                              in1=st[t]["rec_b"][:].unsqueeze(1)
                                  .broadcast_to([P, R_t, H]),
                              op=mybir.AluOpType.mult)
                      # PE: per-edge reciprocal for chunk columns
                      for t in grp:
                          R_t, C_t = plan[t]
                          if C_t == 0:
                              continue
                          j = t - g0
                          ohT = ohT_all[:, CB[t] * P:CB[t + 1] * P]
                          for cc in range(C_t):
                              nc.tensor.matmul(
                                  out=aps[:, RC0 + (j * CCAP + cc) * H:
                                          RC0 + (j * CCAP + cc + 1) * H],
                                  lhsT=ohT[:, cc * P:(cc + 1) * P],
                                  rhs=st[t]["rec_b"][:], start=True,
                                  stop=True)
                      # DVE: normalized alpha, chunk columns
                      for t in grp:
                          R_t, C_t = plan[t]
                          if C_t == 0:
                              continue
                          j = t - g0
                          nc.vector.tensor_tensor(
                              out=exn_all[:, cb0[t] * H:(cb0[t] + C_t) * H],
                              in0=st[t]["ex"][:, 0:C_t * H],
                              in1=aps[:, RC0 + j * CCAP * H:
                                      RC0 + (j * CCAP + C_t) * H],
                              op=mybir.AluOpType.mult)

                  # hx AG after the alpha gathers: the h-loop gathers queue
                  # behind it (they need it), alpha compute overlaps it
                  emit_hx_ag(layer, 0)

                  # ==== h loop: full-row gathers -> messages -> scatter-add ====
                  # one indirect DMA per PAIR of tiles (halves Pool-queue
                  # issue+descgen cost); per-tile views into the pair buffer
                  ch0 = 0
                  gt2 = None
                  for t in range(NT):
                      R_t, C_t = plan[t]
                      ncol = R_t + C_t

                      if t % 2 == 0:
                          pair = [u for u in (t, t + 1) if u < NT]
                          gtot = sum(plan[u][0] + plan[u][1] for u in pair)
                          gt2 = gpool.tile([P, 2 * NCOLMX * HC], BF16,
                                           tag="gt", bufs=2, name="gt")
                          if "no_hgather" in probe:
                              nc.vector.memset(gt2[:, 0:gtot * HC], 0.25)
                          else:
                              nc.gpsimd.indirect_dma_start(
                                  out=gt2[:, 0:gtot * HC], out_offset=None,
                                  in_=hx_full[:],
                                  in_offset=bass.IndirectOffsetOnAxis(
                                      ap=es_all[:, ch0:ch0 + gtot], axis=0),
                              )
                          gt = gt2[:, 0:ncol * HC]
                      else:
                          prev = plan[t - 1][0] + plan[t - 1][1]
                          gt = gt2[:, prev * HC:(prev + ncol) * HC]
                      gtv = gt[:, 0:ncol * HC].rearrange(
                          "p (n c h) -> p n c h", n=ncol, c=C, h=H)

                      msg = gpool.tile([P, NCOLMX * HC], BF16, tag="msg",
                                       bufs=4, name="msg")
                      if "no_msg" in probe:
                          msg = gt
                      else:
                          msgv = msg[:, 0:ncol * HC].rearrange(
                              "p (n c h) -> p n c h", n=ncol, c=C, h=H)
                          exv = exn_all[:, ch0 * H:(ch0 + ncol) * H].rearrange(
                              "p (n h) -> p n h", n=ncol, h=H).unsqueeze(2)\
                              .broadcast_to([P, ncol, C, H])
                          nc.vector.tensor_tensor(out=msgv, in0=gtv, in1=exv,
                                                  op=mybir.AluOpType.mult)

                      num_ps = pspool.tile([P, HC], F32, tag="big", bufs=3,
                                           name="num_ps")
                      sc_cols = [0] if "no_scatter" in probe else list(range(ncol))
                      for cc in sc_cols:
                          lhs = (oh_all[:, (CB[t] + cc) * P:
                                        (CB[t] + cc + 1) * P] if cc < C_t
                                 else identb[:])
                          nc.tensor.matmul(out=num_ps[:], lhsT=lhs,
                                           rhs=msg[:, cc * HC:(cc + 1) * HC],
                                           start=(cc == 0),
                                           stop=(layer == 2
                                                 and cc == sc_cols[-1]))
                      if layer < 2:
                          # bias add: identity @ replicated-bias rows
                          nc.tensor.matmul(out=num_ps[:], lhsT=identb[:],
                                           rhs=b_s[:], start=False, stop=True)

                      if layer < 2:
                          xt = wpool.tile([P, HC], BF16, tag="xt", bufs=4,
                                          name="xt")
                          nc.scalar.activation(xt[:], num_ps[:],
                                               mybir.ActivationFunctionType.Relu)
                          stage_a(xt, layer + 1, t)
                          if t == NT - 1:
                              emit_asrc_ag(layer + 1)
                      else:
                          hm = wpool.tile([P, C], F32, tag="hm", name="hm")
                          nc.vector.tensor_reduce(
                              hm[:],
                              num_ps[:].rearrange("p (c h) -> p c h", c=C, h=H),
                              axis=mybir.AxisListType.X, op=mybir.AluOpType.add)
                          h3f = wpool.tile([P, C], F32, tag="h3f", name="h3f")
                          nc.vector.scalar_tensor_tensor(
                              out=h3f[:], in0=hm[:], scalar=1.0 / H,
                              in1=b3_s[:], op0=mybir.AluOpType.mult,
                              op1=mybir.AluOpType.add)
                          h3 = wpool.tile([P, C], BF16, tag="h3", name="h3")
                          nc.scalar.activation(h3[:], h3f[:],
                                               mybir.ActivationFunctionType.Relu)
                          poh = wpool.tile([P, G], BF16, tag="poh", name="poh")
                          nc.vector.tensor_tensor(
                              out=poh[:],
                              in0=pool_s[:, t:t + 1].to_broadcast([P, G]),
                              in1=iota_f[:, 0:G], op=mybir.AluOpType.is_equal)
                          nc.tensor.matmul(out=pool_ps[:], lhsT=poh[:],
                                           rhs=h3[:], start=(t == 0),
                                           stop=(t == NT - 1))
                      ch0 += ncol

              # ---- pool AllReduce + FC + log_softmax ----
              psb = wpool.tile([G, C], F32)
              nc.vector.tensor_copy(psb[:], pool_ps[:])
              nc.sync.dma_start(out=pool_in[:], in_=psb[:])
              nc.gpsimd.collective_compute(
                  "AllReduce", mybir.AluOpType.add, replica_groups=rg,
                  ins=[pool_in[:]], outs=[pool_out[:]],
              )
              pld = wpool.tile([G, C], F32)
              nc.sync.dma_start(out=pld[:], in_=pool_out[:])
              nc.vector.tensor_scalar_mul(pld[:], pld[:], invc_s[:, 0:1])
              pT_ps = pspool.tile([G, C], F32, tag="small", bufs=3, name="pT_ps")
              nc.tensor.transpose(out=pT_ps[:], in_=pld[:], identity=identf[0:G, 0:C])
              pT = wpool.tile([C, G], F32)
              nc.scalar.copy(pT[:], pT_ps[:])
              z_ps = pspool.tile([G, NCLS], F32, tag="small", bufs=3, name="z_ps")
              nc.tensor.matmul(out=z_ps[:], lhsT=pT[:], rhs=fcw_s[:],
                               start=True, stop=True)
              z = wpool.tile([G, NCLS], F32)
              nc.vector.tensor_add(z[:], z_ps[:], fcb_s[:])
              zm = wpool.tile([G, 1], F32)
              nc.vector.tensor_reduce(zm[:], z[:], axis=mybir.AxisListType.X,
                                      op=mybir.AluOpType.max)
              zs = wpool.tile([G, NCLS], F32)
              nc.vector.tensor_scalar_sub(zs[:], z[:], zm[:, 0:1])
              ze = wpool.tile([G, NCLS], F32)
              nc.scalar.activation(ze[:], zs[:], mybir.ActivationFunctionType.Exp)
              zsum = wpool.tile([G, 1], F32)
              nc.vector.tensor_reduce(zsum[:], ze[:], axis=mybir.AxisListType.X,
                                      op=mybir.AluOpType.add)
              zl = wpool.tile([G, 1], F32)
              nc.scalar.activation(zl[:], zsum[:], mybir.ActivationFunctionType.Ln)
              zo = wpool.tile([G, NCLS], F32)
              nc.vector.tensor_scalar_sub(zo[:], zs[:], zl[:, 0:1])
              nc.sync.dma_start(out=out_ext[:], in_=zo[:])

    nc.compile()
    return nc


def _prepare_inputs(x, edge_index, batch, W1, a1s, a1d, b1, W2, a2s, a2d, b2,
                    W3, a3s, a3d, b3, fcw, fcb):
    plan, esrcT, edstT = _preprocess_edges(
        np.asarray(edge_index, np.int64))
    x = np.asarray(x, np.float32)
    batch = np.asarray(batch, np.int64)

    shared = {
        "w1": _ext_weights(np.asarray(W1, np.float32), np.asarray(a1s, np.float32),
                           np.asarray(a1d, np.float32), perm_rows=False),
        "w2": _ext_weights(np.asarray(W2, np.float32), np.asarray(a2s, np.float32),
                           np.asarray(a2d, np.float32), perm_rows=True),
        "w3": _ext_weights(np.asarray(W3, np.float32), np.asarray(a3s, np.float32),
                           np.asarray(a3d, np.float32), perm_rows=True),
        "b1r": np.tile(np.asarray(b1, np.float32)[_PERM_CM][None, :],
                       (P, 1)).astype(np_bf16),
        "b2r": np.tile(np.asarray(b2, np.float32)[_PERM_CM][None, :],
                       (P, 1)).astype(np_bf16),
        "b3r": np.tile(np.asarray(b3, np.float32)[None, :], (P, 1)).astype(np_bf16),
        "fcw": np.asarray(fcw, np.float32),
        "fcbr": np.tile(np.asarray(fcb, np.float32)[None, :], (G, 1)),
        "invcnt": (1.0 / np.maximum(
            np.bincount(batch, minlength=G), 1.0)).astype(np.float32)[:, None],
    }

    in_maps = []
    for k in range(NCORES):
        xk = np.zeros((NSHP, F_IN), np_bf16)
        xk[:NSH] = x[k * NSH:(k + 1) * NSH].astype(np_bf16)
        pidx = np.full((NSHP,), PAD, np.float32)
        pidx[:NSH] = batch[k * NSH:(k + 1) * NSH]
        poolidx = np.ascontiguousarray(pidx.reshape(NT, P).T)  # [P, NT]
        in_maps.append({
            "x0": xk,
            "esrc": np.ascontiguousarray(esrcT[k]),
            "edst": np.ascontiguousarray(edstT[k]),
            "poolidx": poolidx,
            **shared,
        })
    return plan, in_maps


_CACHE = {}


def _get_nc(plan, repeat=1, probe=frozenset()):
    key = (tuple(plan), repeat, tuple(sorted(probe)))
    if key not in _CACHE:
        _CACHE[key] = _build_nc(plan, repeat, probe)
    return _CACHE[key]


def kernel(x, edge_index, batch, W1, a1s, a1d, b1, W2, a2s, a2d, b2,
           W3, a3s, a3d, b3, fcw, fcb, _trace=False, _results=None):
    plan, in_maps = _prepare_inputs(x, edge_index, batch, W1, a1s, a1d, b1,
                                    W2, a2s, a2d, b2, W3, a3s, a3d, b3, fcw, fcb)
    nc = _get_nc(plan)
    res = run_bass_kernel_spmd(nc, in_maps, core_ids=list(range(NCORES)),
                               trace=_trace)
    if _results is not None:
        _results.append(res)
    return res.results[0]["out"]



# revision 47
# speedup vs baseline: 2.0863x; 1.2472x over previous
"""3-layer GAT (8 heads x 64 ch) + global mean pool + FC + log_softmax on 8 Trainium2 cores.

v2 redesign of the edge phase around batched per-tile operations (vs the
per-column baseline):
- Nodes (and their incoming edges) partitioned across 8 cores; small GAT
  weights replicated. Per layer each core computes [h | a_src] rows (bf16,
  channel-major) for its node shard and AllGathers them into a replicated
  DRAM table; the graph-level mean pool is the only AllReduce.
- Per 128-dst-node tile: ONE wide indirect DMA gathers all edge source rows
  (multi-column offset AP amortizes the ~1us SWDGE descriptor-generation
  cost ~10x), and the attention math runs as a handful of wide strided-AP
  ops. Channel-major rows put the 8 heads innermost so the alpha broadcast
  views keep packed innermost strides (DVE 2x bf16 mode) for the big
  h*alpha multiply.
- alpha is pre-normalized (ex * 1/den) before the message multiply; for
  one-hot overflow chunks the per-edge denominator is fetched through the
  same one-hot matmul used for a_dst. This removes the per-tile
  divide-from-PSUM and lets the layer bias fold into the PSUM accumulation
  as an identity-matmul of replicated bias rows, so the finalize is a
  single Relu activation straight out of PSUM.
- Engine placement: Pool runs the gathers + SBUF-only logit adds; DVE the
  one-hot builds, denominators and the msg multiply; Act exp/copies/relu;
  PE transposes, scatter-add and the fused next-layer x@W (stage A), which
  hides under the gather-DMA phase. bf16 matmuls run at 1 cycle/row (fp32
  is 4).
- `repeat=` builds the body K times (fresh DRAM tables per rep) purely for
  timing: (T(K)-T(1))/(K-1) cancels the ~80ms axon RPC floor.
"""

import numpy as np
from ml_dtypes import bfloat16 as np_bf16

import concourse.bass as bass
import concourse.mybir as mybir
import concourse.tile as tile
from concourse import bacc
from concourse.bass_utils import run_bass_kernel_spmd
from concourse.masks import make_identity

# problem constants (hardcoded per contract)
N, E, F_IN, H, C, G, NCLS = 50000, 400000, 128, 8, 64, 64, 10
HC = H * C  # 512
NEG = 0.2
EPS = 1e-16

NCORES = 8
P = 128
NSH = N // NCORES          # 6250 nodes per core
NT = (NSH + P - 1) // P    # 49 dst tiles per core
NSHP = NT * P              # 6272 padded rows per core
NFULL = NCORES * NSHP      # 50176 rows in the gathered table
ROWW = HC + H              # 520: h (chan-major) | a_src
WEXT = HC + 2 * H          # 528: W | Wa_src | Wa_dst
PAD = 999.0                # one-hot miss marker for padded overflow slots
MASKNEG = -1.0e30          # a_src value planted in the pad row (=> exp -> 0)
CCAP = 4                   # max one-hot chunk columns per tile (PSUM bank)
NCAP = 16                  # max total columns per tile (SBUF budget)
# hx AllGather split boundaries (tile granularity). A Shared DRAM tensor
# may only be written by one instruction, so the hx AG is a single segment.
SPLIT_TILES = (0, 49)
SPLIT_ROWS = tuple(t * P for t in SPLIT_TILES)
# pad slots gather the pad row (core 0, local row NSHP-1) whose a_src holds
# MASKNEG, so padded round edges contribute exp(-inf)=0 without a mask op.
PADGID_H = NCORES * SPLIT_ROWS[-2] + (NSHP - 1 - SPLIT_ROWS[-2])
PADGID_A = NSHP - 1
PADSTART = NSH - (NT - 1) * P  # first pad partition in the last tile
GRP = 7                    # alpha-loop stage-major group size (tiles)

F32 = mybir.dt.float32
BF16 = mybir.dt.bfloat16
I32 = mybir.dt.int32


def _preprocess_edges(edge_index):
    """Assign edges (incl. self loops) to the dst-owning core; build per-tile
    round columns (node's r-th edge in its own partition) plus overflow
    one-hot chunk columns.

    Returns (plan, esrcT, edstT):
      plan: list of (R_t, C_t) per tile — shared by all cores.
      esrcT[k] int32 [P, TOTCOL]: hx gather row ids, seg-major per
        SPLIT_ROWS (pad -> PADGID_H, whose a_src holds MASKNEG).
      edstT[k] f32 [P, TOTCOL]: chunk cols -> dst slot or PAD (round cols
        unused by the kernel)."""
    src = np.concatenate([edge_index[0], np.arange(N, dtype=np.int64)])
    dst = np.concatenate([edge_index[1], np.arange(N, dtype=np.int64)])
    core = dst // NSH
    dloc = dst - core * NSH
    tile_of = dloc // P
    slot = dloc - tile_of * P
    # hx gather table layout: SPLITS segments, each rank-major over its rows
    sk = src // NSH
    sr = src % NSH
    split_rows = np.asarray(SPLIT_ROWS, np.int64)
    seg = np.searchsorted(split_rows[1:], sr, side="right")
    r0 = split_rows[seg]
    r1 = split_rows[seg + 1]
    gid = NCORES * r0 + sk * (r1 - r0) + (sr - r0)

    deg = np.zeros((NCORES, NT, P), np.int64)
    np.add.at(deg, (core, tile_of, slot), 1)
    maxdeg_t = deg.max(axis=(0, 2))  # [NT]

    # choose the round cap per tile: chunk columns carry extra one-hot work
    R_ts = np.zeros(NT, np.int64)
    C_ts = np.zeros(NT, np.int64)
    for t in range(NT):
        best = None
        for rcap in range(1, int(maxdeg_t[t]) + 1):
            r = min(int(maxdeg_t[t]), rcap)
            ovf = np.maximum(deg[:, t, :] - rcap, 0).sum(axis=1)
            c = int(np.ceil(ovf / P).max())
            if c > CCAP or r + c > NCAP:
                continue
            cost = r + 1.35 * c
            if best is None or cost < best[0]:
                best = (cost, r, c)
        assert best is not None, f"no feasible plan for tile {t}"
        _, R_ts[t], C_ts[t] = best
    plan = [(int(R_ts[t]), int(C_ts[t])) for t in range(NT)]
    colbase = np.zeros(NT, np.int64)
    colbase[1:] = np.cumsum(R_ts + C_ts)[:-1]
    TOTCOL = int((R_ts + C_ts).sum())

    esrcT = np.full((NCORES, P, TOTCOL), PADGID_H, np.int32)
    edstT = np.empty((NCORES, P, TOTCOL), np.float32)
    for k in range(NCORES):
        # default fill: chunks PAD (rounds unused)
        for t in range(NT):
            b = colbase[t]
            edstT[k, :, b:b + C_ts[t]] = PAD
            edstT[k, :, b + C_ts[t]:b + C_ts[t] + R_ts[t]] = 0.0
        m = core == k
        t_k, s_k, g_k = tile_of[m], slot[m], gid[m]
        order = np.argsort(t_k * P + s_k, kind="stable")
        t_k, s_k, g_k = t_k[order], s_k[order], g_k[order]
        node = t_k * P + s_k
        start = np.zeros(NT * P + 1, np.int64)
        np.add.at(start[1:], node, 1)
        start = np.cumsum(start)
        j = np.arange(len(node)) - start[node]  # rank within node
        rmax = R_ts[t_k]
        isr = j < rmax
        # round entries (after the C_t chunk columns)
        rcol = colbase[t_k[isr]] + C_ts[t_k[isr]] + j[isr]
        esrcT[k, s_k[isr], rcol] = g_k[isr].astype(np.int32)
        # overflow entries: sequential position within each tile
        to, so, go = t_k[~isr], s_k[~isr], g_k[~isr]
        oorder = np.argsort(to, kind="stable")
        to, so, go = to[oorder], so[oorder], go[oorder]
        ostart = np.zeros(NT + 1, np.int64)
        np.add.at(ostart[1:], to, 1)
        ostart = np.cumsum(ostart)
        q = np.arange(len(to)) - ostart[to]
        col = colbase[to] + q // P
        row = q % P
        esrcT[k, row, col] = go.astype(np.int32)
        edstT[k, row, col] = so.astype(np.float32)
    return plan, esrcT, edstT


# channel-major permutation: cm index (c*H + h) <- standard (h*C + c)
_PERM_CM = np.array([h * C + c for c in range(C) for h in range(H)], np.int64)


def _ext_weights(W, a_s, a_d, perm_rows):
    """bf16 [P, nk, 528] = [W_cm | W@A_s | W@A_d]; columns chan-major, rows
    permuted chan-major when the layer input is chan-major."""
    K = W.shape[0]
    if perm_rows:
        W = W[_PERM_CM, :]
    Wr = W.reshape(K, H, C)
    ws = np.einsum("fhc,hc->fh", Wr, a_s)
    wd = np.einsum("fhc,hc->fh", Wr, a_d)
    Wx = np.concatenate([W[:, _PERM_CM], ws, wd], axis=1).astype(np.float32)
    nk = K // P
    return np.ascontiguousarray(
        Wx.reshape(nk, P, WEXT).transpose(1, 0, 2)).astype(np_bf16)


def _build_nc(plan, repeat=1, probe=frozenset()):
    probe = frozenset(probe)
    TOTCOL = sum(r + c for r, c in plan)
    NCOLMX = max(r + c for r, c in plan)
    CMX = max(c for _, c in plan)
    nc = bacc.Bacc("TRN2", target_bir_lowering=False, debug=False,
                   num_devices=NCORES)

    x_ext = nc.dram_tensor("x0", [NSHP, F_IN], BF16, kind="ExternalInput")
    esrc_ext = nc.dram_tensor("esrc", [P, TOTCOL], I32, kind="ExternalInput")
    edst_ext = nc.dram_tensor("edst", [P, TOTCOL], F32, kind="ExternalInput")
    w1_ext = nc.dram_tensor("w1", [P, 1, WEXT], BF16, kind="ExternalInput")
    w2_ext = nc.dram_tensor("w2", [P, 4, WEXT], BF16, kind="ExternalInput")
    w3_ext = nc.dram_tensor("w3", [P, 4, WEXT], BF16, kind="ExternalInput")
    b1_ext = nc.dram_tensor("b1r", [P, HC], BF16, kind="ExternalInput")
    b2_ext = nc.dram_tensor("b2r", [P, HC], BF16, kind="ExternalInput")
    b3_ext = nc.dram_tensor("b3r", [P, C], BF16, kind="ExternalInput")
    pool_ext = nc.dram_tensor("poolidx", [P, NT], F32, kind="ExternalInput")
    invc_ext = nc.dram_tensor("invcnt", [G, 1], F32, kind="ExternalInput")
    fcw_ext = nc.dram_tensor("fcw", [C, NCLS], F32, kind="ExternalInput")
    fcb_ext = nc.dram_tensor("fcbr", [G, NCLS], F32, kind="ExternalInput")
    out_ext = nc.dram_tensor("out", [G, NCLS], F32, kind="ExternalOutput")

    rg = [list(range(NCORES))]

    with tile.TileContext(nc) as tc:
        with (
            tc.tile_pool(name="const", bufs=1) as cpool,
            tc.tile_pool(name="work", bufs=3) as wpool,
            tc.tile_pool(name="gat", bufs=3) as gpool,
            tc.tile_pool(name="ps", bufs=1, space="PSUM") as pspool,
            tc.tile_pool(name="dram", bufs=1, space="DRAM") as dpool,
        ):
            # ---- constants ----
            iota_i = cpool.tile([P, P], I32)
            nc.gpsimd.iota(iota_i[:], pattern=[[1, P]], base=0, channel_multiplier=0)
            iota_f = cpool.tile([P, P], F32)
            nc.vector.tensor_copy(iota_f[:], iota_i[:])
            identf = cpool.tile([P, P], F32)
            make_identity(nc, identf[:])
            identb = cpool.tile([P, P], BF16)
            make_identity(nc, identb[:])

            w1_s = cpool.tile([P, 1, WEXT], BF16)
            nc.sync.dma_start(out=w1_s[:], in_=w1_ext[:])
            w2_s = cpool.tile([P, 4, WEXT], BF16)
            nc.sync.dma_start(out=w2_s[:], in_=w2_ext[:])
            w3_s = cpool.tile([P, 4, WEXT], BF16)
            nc.sync.dma_start(out=w3_s[:], in_=w3_ext[:])
            b1_s = cpool.tile([P, HC], BF16)
            nc.sync.dma_start(out=b1_s[:], in_=b1_ext[:])
            b2_s = cpool.tile([P, HC], BF16)
            nc.sync.dma_start(out=b2_s[:], in_=b2_ext[:])
            b3_s = cpool.tile([P, C], BF16)
            nc.sync.dma_start(out=b3_s[:], in_=b3_ext[:])
            pool_s = cpool.tile([P, NT], F32)
            nc.sync.dma_start(out=pool_s[:], in_=pool_ext[:])
            invc_s = cpool.tile([G, 1], F32)
            nc.sync.dma_start(out=invc_s[:], in_=invc_ext[:])
            fcw_s = cpool.tile([C, NCLS], F32)
            nc.sync.dma_start(out=fcw_s[:], in_=fcw_ext[:])
            fcb_s = cpool.tile([G, NCLS], F32)
            nc.sync.dma_start(out=fcb_s[:], in_=fcb_ext[:])
            es_all = cpool.tile([P, TOTCOL], I32)
            nc.sync.dma_start(out=es_all[:], in_=esrc_ext[:])
            ed_all = cpool.tile([P, TOTCOL], F32)
            nc.sync.dma_start(out=ed_all[:], in_=edst_ext[:])
            # pad-partition mask for the last tile: rows >= PADSTART get
            # MASKNEG added to their a_src so pad-row gathers vanish pre-exp
            pidx_i = cpool.tile([P, 1], I32)
            nc.gpsimd.iota(pidx_i[:], pattern=[[0, 1]], base=0,
                           channel_multiplier=1)
            pidx_f = cpool.tile([P, 1], F32)
            nc.vector.tensor_copy(pidx_f[:], pidx_i[:])
            padmask = cpool.tile([P, 1], F32)
            nc.vector.tensor_scalar(padmask[:], pidx_f[:], PADSTART - 0.5,
                                    MASKNEG, op0=mybir.AluOpType.is_ge,
                                    op1=mybir.AluOpType.mult)
            # per-node a_dst for current/next layer (f32 adds + bf16 matmul rhs)
            adf_a = cpool.tile([P, NT * H], F32)
            adf_b = cpool.tile([P, NT * H], F32)
            adb_a = cpool.tile([P, NT * H], BF16)
            adb_b = cpool.tile([P, NT * H], BF16)

            # static one-hot tables for all chunk columns (edge data only)
            CB = [0]
            for _r, _c in plan:
                CB.append(CB[-1] + _c)
            TCH = CB[-1]
            if TCH > 0:
                oh_all = cpool.tile([P, TCH * P], BF16)
                ohT_all = cpool.tile([P, TCH * P], BF16)
                _ch0 = 0
                for _t in range(NT):
                    _R, _Ct = plan[_t]
                    if _Ct > 0:
                        _edv = ed_all[:, _ch0:_ch0 + _Ct].unsqueeze(2)\
                            .broadcast_to([P, _Ct, P])
                        _iov = iota_f[:].unsqueeze(1).broadcast_to([P, _Ct, P])
                        nc.vector.tensor_tensor(
                            out=oh_all[:, CB[_t] * P:CB[_t + 1] * P].rearrange(
                                "p (n q) -> p n q", n=_Ct, q=P),
                            in0=_edv, in1=_iov, op=mybir.AluOpType.is_equal)
                        _ohT_ps = pspool.tile([P, CCAP * P], BF16, tag="trans",
                                              bufs=2, name="ohT_ps")
                        for _cc in range(_Ct):
                            nc.tensor.transpose(
                                out=_ohT_ps[:, _cc * P:(_cc + 1) * P],
                                in_=oh_all[:, (CB[_t] + _cc) * P:
                                           (CB[_t] + _cc + 1) * P],
                                identity=identb[:])
                        nc.scalar.copy(ohT_all[:, CB[_t] * P:CB[_t + 1] * P],
                                       _ohT_ps[:, 0:_Ct * P])
                    _ch0 += _R + _Ct

            hx_local = hx_fulls = pool_in = pool_out = None

            w_tiles = (w1_s, w2_s, w3_s)
            b_tiles = (b1_s, b2_s, b3_s)
            adf_of = (adf_a, adf_b, adf_a)
            adb_of = (adb_a, adb_b, adb_a)
            split_end = {SPLIT_TILES[i + 1] - 1: i
                         for i in range(len(SPLIT_TILES) - 1)}

            # The asrc AG is emitted at the end of the producing loop; the
            # big hx AG is emitted AFTER the next alpha loop's gathers so
            # those Pool-queue gathers are not parked behind its transfer,
            # and the alpha compute overlaps the hx AG.
            def emit_asrc_ag(layer):
                if "no_ag" in probe:
                    return
                nc.gpsimd.collective_compute(
                    "AllGather", mybir.AluOpType.bypass, replica_groups=rg,
                    ins=[asrc_local[:]],
                    outs=[asrc_fulls[layer][:]],
                )

            def emit_hx_ag(layer, seg):
                if "no_ag" in probe:
                    return
                r0, r1 = SPLIT_ROWS[seg], SPLIT_ROWS[seg + 1]
                go = NCORES * r0
                nc.gpsimd.collective_compute(
                    "AllGather", mybir.AluOpType.bypass, replica_groups=rg,
                    ins=[hx_local[r0:r1, :]],
                    outs=[hx_fulls[layer][go:go + NCORES * (r1 - r0), :]],
                )

            def stage_a(xt, layer, t, dst=None, write_ad=True,
                        split_copy=False):
                """xt: SBUF bf16 [P, K] node-tile features for `layer`; emits
                [h | a_src] -> dst rows (default hx_local) and a_dst -> adf/adb."""
                K = F_IN if layer == 0 else HC
                nk = K // P
                w_s = w_tiles[layer]
                xT_ps = pspool.tile([P, HC], BF16, tag="trans", bufs=2,
                                    name="xT_ps")
                for j in range(nk):
                    nc.tensor.transpose(out=xT_ps[:, j * P:(j + 1) * P],
                                        in_=xt[:, j * P:(j + 1) * P],
                                        identity=identb[:])
                xT = wpool.tile([P, HC], BF16, tag="xT", bufs=4, name="xT")
                nc.scalar.copy(xT[:, 0:K], xT_ps[:, 0:K])
                h_ps = pspool.tile([P, HC], F32, tag="big", bufs=3, name="h_ps")
                a_ps = pspool.tile([P, 2 * H], F32, tag="small", bufs=3,
                                   name="a_ps")
                for j in range(nk):
                    nc.tensor.matmul(out=h_ps[:], lhsT=xT[:, j * P:(j + 1) * P],
                                     rhs=w_s[:, j, 0:HC],
                                     start=(j == 0), stop=(j == nk - 1))
                    nc.tensor.matmul(out=a_ps[:], lhsT=xT[:, j * P:(j + 1) * P],
                                     rhs=w_s[:, j, HC:WEXT],
                                     start=(j == 0), stop=(j == nk - 1))
                hx_t = wpool.tile([P, HC], BF16, tag="hx_t", bufs=4, name="hx_t")
                if split_copy:
                    nc.scalar.copy(hx_t[:, 0:HC // 2], h_ps[:, 0:HC // 2])
                    nc.vector.tensor_copy(hx_t[:, HC // 2:HC], h_ps[:, HC // 2:HC])
                else:
                    nc.scalar.copy(hx_t[:, 0:HC], h_ps[:])
                hxa_t = wpool.tile([P, H], BF16, tag="hxa_t", bufs=4,
                                   name="hxa_t")
                if t == NT - 1:
                    # plant MASKNEG in the pad rows' a_src (pad-slot target)
                    nc.vector.tensor_tensor(
                        out=hxa_t[:], in0=a_ps[:, 0:H],
                        in1=padmask[:, 0:1].to_broadcast([P, H]),
                        op=mybir.AluOpType.add)
                else:
                    nc.vector.tensor_copy(hxa_t[:], a_ps[:, 0:H])
                nc.sync.dma_start(out=asrc_local[t * P:(t + 1) * P, :],
                                  in_=hxa_t[:])
                if write_ad:
                    adf_n = adf_of[layer]
                    adb_n = adb_of[layer]
                    nc.vector.tensor_copy(adf_n[:, t * H:(t + 1) * H],
                                          a_ps[:, H:2 * H])
                    nc.vector.tensor_copy(adb_n[:, t * H:(t + 1) * H],
                                          a_ps[:, H:2 * H])
                if dst is None:
                    dst = hx_local
                nc.sync.dma_start(out=dst[t * P:(t + 1) * P, :], in_=hx_t[:])

            # ---- layer-0 stage A (from input features) ----
            for _rep in range(repeat):
              hx_local = dpool.tile([NSHP, HC], BF16, name="hx_local")
              asrc_local = dpool.tile([NSHP, H], BF16, name="asrc_local")
              hx_fulls = [
                  dpool.tile([NFULL, HC], BF16, addr_space="Shared",
                             name=f"hx_full{i}")
                  for i in range(3)
              ]
              asrc_fulls = [
                  dpool.tile([NFULL, H], BF16, addr_space="Shared",
                             name=f"asrc_full{i}")
                  for i in range(3)
              ]
              pool_in = dpool.tile([G, C], F32, name="pool_in")
              pool_out = dpool.tile([G, C], F32, addr_space="Shared",
                                    name="pool_out")
              for t in range(NT):
                  xt_b = wpool.tile([P, F_IN], BF16, tag="xt0", name="xt0")
                  nc.sync.dma_start(out=xt_b[:], in_=x_ext[t * P:(t + 1) * P, :])
                  stage_a(xt_b, 0, t, split_copy=True)
                  if t == NT - 1:
                      emit_asrc_ag(0)

              pool_ps = None
              for layer in range(3):
                  hx_full = hx_fulls[layer]
                  asrc_full = asrc_fulls[layer]
                  b_s = b_tiles[layer]
                  adf_cur = adf_of[layer]
                  adb_cur = adb_of[layer]
                  if layer == 2:
                      pool_ps = pspool.tile([G, C], F32, tag="small", bufs=3,
                                            name="pool_ps")

                  # ==== alpha loop: stage-major tile groups =================
                  # Per group of GRP tiles each stage is emitted for every
                  # tile before the next stage, so each engine gets long
                  # same-stage instruction runs and cross-engine semaphore
                  # waits amortize over the group instead of per tile.
                  # Per-group PSUM scratch packs [adpe | den | recpe] into a
                  # single bank-sized tile (PSUM slots are bank-granular and
                  # all 8 banks are claimed by existing tags).
                  exn_all = wpool.tile([P, TOTCOL * H], BF16, tag="exnall",
                                       bufs=2, name="exn_all")
                  if "no_alpha_compute" in probe:
                      nc.vector.memset(exn_all[:], 0.25)
                  cb0 = []
                  ch0 = 0
                  for t in range(NT):
                      cb0.append(ch0)
                      ch0 += plan[t][0] + plan[t][1]
                  AD0 = 0                    # adpe region base (f32 cols)
                  DN0 = GRP * CCAP * H       # den region base
                  RC0 = DN0 + GRP * H        # recpe region base
                  for g0 in range(0, NT, GRP):
                      grp = list(range(g0, min(g0 + GRP, NT)))
                      st = {t: {} for t in grp}
                      aps = pspool.tile([P, RC0 + GRP * CCAP * H], F32,
                                        tag="small", bufs=3, name="alpha_ps")
                      # PE: a_dst for chunk edges via one-hot (needs adb only)
                      if "no_alpha_compute" not in probe:
                          for j, t in enumerate(grp):
                              R_t, C_t = plan[t]
                              if C_t == 0:
                                  continue
                              adb = adb_cur[:, t * H:(t + 1) * H]
                              ohT = ohT_all[:, CB[t] * P:CB[t + 1] * P]
                              for cc in range(C_t):
                                  nc.tensor.matmul(
                                      out=aps[:, AD0 + (j * CCAP + cc) * H:
                                              AD0 + (j * CCAP + cc + 1) * H],
                                      lhsT=ohT[:, cc * P:(cc + 1) * P],
                                      rhs=adb, start=True, stop=True)
                      # gpsimd: ONE batched a_src strip gather per group
                      gcb = cb0[grp[0]]
                      gcols = sum(plan[t][0] + plan[t][1] for t in grp)
                      ga_g = gpool.tile([P, GRP * NCOLMX * H], BF16, tag="ga",
                                        bufs=2, name="ga")
                      if "no_agather" in probe:
                          nc.vector.memset(ga_g[:, 0:gcols * H], 0.25)
                      else:
                          nc.gpsimd.indirect_dma_start(
                              out=ga_g[:, 0:gcols * H], out_offset=None,
                              in_=asrc_full[:],
                              in_offset=bass.IndirectOffsetOnAxis(
                                  ap=es_all[:, gcb:gcb + gcols], axis=0),
                          )
                      for t in grp:
                          ncol = plan[t][0] + plan[t][1]
                          st[t]["ga"] = ga_g[:, (cb0[t] - gcb) * H:
                                             (cb0[t] - gcb + ncol) * H]
                      if "no_alpha_compute" in probe:
                          continue
                      # DVE: logits (pad slots arrive as MASKNEG) + leaky relu
                      for t in grp:
                          R_t, C_t = plan[t]
                          ncol = R_t + C_t
                          j = t - g0
                          ga = st[t]["ga"]
                          adf = adf_cur[:, t * H:(t + 1) * H]
                          logit = wpool.tile([P, NCOLMX * H], F32, tag="logit",
                                             bufs=GRP + 1, name="logit")
                          lgv = logit[:, 0:ncol * H].rearrange(
                              "p (n h) -> p n h", n=ncol, h=H)
                          nc.vector.tensor_tensor(
                              out=lgv[:, C_t:ncol, :],
                              in0=ga[:, C_t * H:ncol * H].rearrange(
                                  "p (n h) -> p n h", n=R_t, h=H),
                              in1=adf[:].unsqueeze(1).broadcast_to(
                                  [P, R_t, H]),
                              op=mybir.AluOpType.add)
                          if C_t > 0:
                              nc.vector.tensor_tensor(
                                  out=lgv[:, 0:C_t, :],
                                  in0=ga[:, 0:C_t * H].rearrange(
                                      "p (n h) -> p n h", n=C_t, h=H),
                                  in1=aps[:, AD0 + j * CCAP * H:
                                          AD0 + (j * CCAP + C_t) * H]
                                      .rearrange("p (n h) -> p n h",
                                                 n=C_t, h=H),
                                  op=mybir.AluOpType.add)
                          lr = wpool.tile([P, NCOLMX * H], F32, tag="lr",
                                          bufs=GRP + 1, name="lr")
                          nc.vector.scalar_tensor_tensor(
                              out=lr[:, 0:ncol * H], in0=logit[:, 0:ncol * H],
                              scalar=NEG, in1=logit[:, 0:ncol * H],
                              op0=mybir.AluOpType.mult,
                              op1=mybir.AluOpType.max)
                          st[t]["lr"] = lr
                      # Act: exp (straight to bf16)
                      for t in grp:
                          ncol = plan[t][0] + plan[t][1]
                          ex = wpool.tile([P, NCOLMX * H], BF16, tag="ex",
                                          bufs=GRP + 1, name="ex")
                          nc.scalar.activation(
                              ex[:, 0:ncol * H], st[t]["lr"][:, 0:ncol * H],
                              mybir.ActivationFunctionType.Exp)
                          st[t]["ex"] = ex
                      # DVE: round-column denominator
                      for t in grp:
                          R_t, C_t = plan[t]
                          ncol = R_t + C_t
                          den_r = wpool.tile([P, H], F32, tag="den_r",
                                             bufs=GRP + 1, name="den_r")
                          exrv = st[t]["ex"][:, C_t * H:ncol * H].rearrange(
                              "p (n h) -> p n h", n=R_t, h=H)\
                              .transpose([0, 2, 1])
                          nc.vector.tensor_reduce(den_r[:], exrv,
                                                  axis=mybir.AxisListType.X,
                                                  op=mybir.AluOpType.add)
                          st[t]["den_r"] = den_r
                      # PE: chunk-column denominator via one-hot
                      for t in grp:
                          R_t, C_t = plan[t]
                          if C_t == 0:
                              continue
                          j = t - g0
                          oh = oh_all[:, CB[t] * P:CB[t + 1] * P]
                          for cc in range(C_t):
                              nc.tensor.matmul(
                                  out=aps[:, DN0 + j * H:DN0 + (j + 1) * H],
                                  lhsT=oh[:, cc * P:(cc + 1) * P],
                                  rhs=st[t]["ex"][:, cc * H:(cc + 1) * H],
                                  start=(cc == 0), stop=(cc == C_t - 1))
                      # DVE: total denominator + reciprocal (bf16)
                      for t in grp:
                          R_t, C_t = plan[t]
                          j = t - g0
                          rec_b = wpool.tile([P, H], BF16, tag="rec_b",
                                             bufs=GRP + 1, name="rec_b")
                          den = wpool.tile([P, H], F32, tag="den",
                                           bufs=GRP + 1, name="den")
                          if C_t > 0:
                              # EPS keeps pad nodes (no real edges) finite
                              nc.vector.scalar_tensor_tensor(
                                  out=den[:], in0=st[t]["den_r"][:],
                                  scalar=EPS,
                                  in1=aps[:, DN0 + j * H:DN0 + (j + 1) * H],
                                  op0=mybir.AluOpType.add,
                                  op1=mybir.AluOpType.add)
                          else:
                              nc.vector.tensor_scalar_add(
                                  den[:], st[t]["den_r"][:], EPS)
                          with nc.allow_low_precision(
                                  reason="1/den to bf16; alpha tol ~1e-2"):
                              nc.vector.reciprocal(rec_b[:], den[:])
                          st[t]["rec_b"] = rec_b
                      # DVE: normalized alpha, round columns
                      for t in grp:
                          R_t, C_t = plan[t]
                          ncol = R_t + C_t
                          nc.vector.tensor_tensor(
                              out=exn_all[:, (cb0[t] + C_t) * H:
                                          (cb0[t] + ncol) * H].rearrange(
                                  "p (n h) -> p n h", n=R_t, h=H),
                              in0=st[t]["ex"][:, C_t * H:ncol * H].rearrange(
                                  "p (n h) -> p n h", n=R_t, h=H),
                              in1=st[t]["rec_b"][:].unsqueeze(1)
                                  .broadcast_to([P, R_t, H]),
                              op=mybir.AluOpType.mult)
                      # PE: per-edge reciprocal for chunk columns
                      for t in grp:
                          R_t, C_t = plan[t]
                          if C_t == 0:
                              continue
                          j = t - g0
                          ohT = ohT_all[:, CB[t] * P:CB[t + 1] * P]
                          for cc in range(C_t):
                              nc.tensor.matmul(
                                  out=aps[:, RC0 + (j * CCAP + cc) * H:
                                          RC0 + (j * CCAP + cc + 1) * H],
                                  lhsT=ohT[:, cc * P:(cc + 1) * P],
                                  rhs=st[t]["rec_b"][:], start=True,
                                  stop=True)
                      # DVE: normalized alpha, chunk columns
                      for t in grp:
                          R_t, C_t = plan[t]
                          if C_t == 0:
                              continue
                          j = t - g0
                          nc.vector.tensor_tensor(
                              out=exn_all[:, cb0[t] * H:(cb0[t] + C_t) * H],
                              in0=st[t]["ex"][:, 0:C_t * H],
                              in1=aps[:, RC0 + j * CCAP * H:
                                      RC0 + (j * CCAP + C_t) * H],
                              op=mybir.AluOpType.mult)

                  # hx AG after the alpha gathers: the h-loop gathers queue
                  # behind it (they need it), alpha compute overlaps it
                  emit_hx_ag(layer, 0)

                  # ==== h loop: full-row gathers -> messages -> scatter-add ====
                  ch0 = 0
                  for t in range(NT):
                      R_t, C_t = plan[t]
                      ncol = R_t + C_t

                      gt = gpool.tile([P, NCOLMX * HC], BF16, tag="gt", bufs=4,
                                      name="gt")
                      if "no_hgather" in probe:
                          nc.vector.memset(gt[:, 0:ncol * HC], 0.25)
                      else:
                          nc.gpsimd.indirect_dma_start(
                              out=gt[:, 0:ncol * HC], out_offset=None,
                              in_=hx_full[:],
                              in_offset=bass.IndirectOffsetOnAxis(
                                  ap=es_all[:, ch0:ch0 + ncol], axis=0),
                          )
                      gtv = gt[:, 0:ncol * HC].rearrange(
                          "p (n c h) -> p n c h", n=ncol, c=C, h=H)

                      msg = gpool.tile([P, NCOLMX * HC], BF16, tag="msg",
                                       bufs=4, name="msg")
                      if "no_msg" in probe:
                          msg = gt
                      else:
                          msgv = msg[:, 0:ncol * HC].rearrange(
                              "p (n c h) -> p n c h", n=ncol, c=C, h=H)
                          exv = exn_all[:, ch0 * H:(ch0 + ncol) * H].rearrange(
                              "p (n h) -> p n h", n=ncol, h=H).unsqueeze(2)\
                              .broadcast_to([P, ncol, C, H])
                          nc.vector.tensor_tensor(out=msgv, in0=gtv, in1=exv,
                                                  op=mybir.AluOpType.mult)

                      num_ps = pspool.tile([P, HC], F32, tag="big", bufs=3,
                                           name="num_ps")
                      sc_cols = [0] if "no_scatter" in probe else list(range(ncol))
                      for cc in sc_cols:
                          lhs = (oh_all[:, (CB[t] + cc) * P:
                                        (CB[t] + cc + 1) * P] if cc < C_t
                                 else identb[:])
                          nc.tensor.matmul(out=num_ps[:], lhsT=lhs,
                                           rhs=msg[:, cc * HC:(cc + 1) * HC],
                                           start=(cc == 0),
                                           stop=(layer == 2
                                                 and cc == sc_cols[-1]))
                      if layer < 2:
                          # bias add: identity @ replicated-bias rows
                          nc.tensor.matmul(out=num_ps[:], lhsT=identb[:],
                                           rhs=b_s[:], start=False, stop=True)

                      if layer < 2:
                          xt = wpool.tile([P, HC], BF16, tag="xt", bufs=4,
                                          name="xt")
                          nc.scalar.activation(xt[:], num_ps[:],
                                               mybir.ActivationFunctionType.Relu)
                          stage_a(xt, layer + 1, t)
                          if t == NT - 1:
                              emit_asrc_ag(layer + 1)
                      else:
                          hm = wpool.tile([P, C], F32, tag="hm", name="hm")
                          nc.vector.tensor_reduce(
                              hm[:],
                              num_ps[:].rearrange("p (c h) -> p c h", c=C, h=H),
                              axis=mybir.AxisListType.X, op=mybir.AluOpType.add)
                          h3f = wpool.tile([P, C], F32, tag="h3f", name="h3f")
                          nc.vector.scalar_tensor_tensor(
                              out=h3f[:], in0=hm[:], scalar=1.0 / H,
                              in1=b3_s[:], op0=mybir.AluOpType.mult,
                              op1=mybir.AluOpType.add)
                          h3 = wpool.tile([P, C], BF16, tag="h3", name="h3")
                          nc.scalar.activation(h3[:], h3f[:],
                                               mybir.ActivationFunctionType.Relu)
                          poh = wpool.tile([P, G], BF16, tag="poh", name="poh")
                          nc.vector.tensor_tensor(
                              out=poh[:],
                              in0=pool_s[:, t:t + 1].to_broadcast([P, G]),
                              in1=iota_f[:, 0:G], op=mybir.AluOpType.is_equal)
                          nc.tensor.matmul(out=pool_ps[:], lhsT=poh[:],
                                           rhs=h3[:], start=(t == 0),
                                           stop=(t == NT - 1))
                      ch0 += ncol

              # ---- pool AllReduce + FC + log_softmax ----
              psb = wpool.tile([G, C], F32)
              nc.vector.tensor_copy(psb[:], pool_ps[:])
              nc.sync.dma_start(out=pool_in[:], in_=psb[:])
              nc.gpsimd.collective_compute(
                  "AllReduce", mybir.AluOpType.add, replica_groups=rg,
                  ins=[pool_in[:]], outs=[pool_out[:]],
              )
              pld = wpool.tile([G, C], F32)
              nc.sync.dma_start(out=pld[:], in_=pool_out[:])
              nc.vector.tensor_scalar_mul(pld[:], pld[:], invc_s[:, 0:1])
              pT_ps = pspool.tile([G, C], F32, tag="small", bufs=3, name="pT_ps")
              nc.tensor.transpose(out=pT_ps[:], in_=pld[:], identity=identf[0:G, 0:C])
              pT = wpool.tile([C, G], F32)
              nc.scalar.copy(pT[:], pT_ps[:])
              z_ps = pspool.tile([G, NCLS], F32, tag="small", bufs=3, name="z_ps")
              nc.tensor.matmul(out=z_ps[:], lhsT=pT[:], rhs=fcw_s[:],
                               start=True, stop=True)
              z = wpool.tile([G, NCLS], F32)
              nc.vector.tensor_add(z[:], z_ps[:], fcb_s[:])
              zm = wpool.tile([G, 1], F32)
              nc.vector.tensor_reduce(zm[:], z[:], axis=mybir.AxisListType.X,
                                      op=mybir.AluOpType.max)
              zs = wpool.tile([G, NCLS], F32)
              nc.vector.tensor_scalar_sub(zs[:], z[:], zm[:, 0:1])
              ze = wpool.tile([G, NCLS], F32)
              nc.scalar.activation(ze[:], zs[:], mybir.ActivationFunctionType.Exp)
              zsum = wpool.tile([G, 1], F32)
              nc.vector.tensor_reduce(zsum[:], ze[:], axis=mybir.AxisListType.X,
                                      op=mybir.AluOpType.add)
              zl = wpool.tile([G, 1], F32)
              nc.scalar.activation(zl[:], zsum[:], mybir.ActivationFunctionType.Ln)
              zo = wpool.tile([G, NCLS], F32)
              nc.vector.tensor_scalar_sub(zo[:], zs[:], zl[:, 0:1])
              nc.sync.dma_start(out=out_ext[:], in_=zo[:])

    nc.compile()
    return nc


def _prepare_inputs(x, edge_index, batch, W1, a1s, a1d, b1, W2, a2s, a2d, b2,
                    W3, a3s, a3d, b3, fcw, fcb):
    plan, esrcT, edstT = _preprocess_edges(
        np.asarray(edge_index, np.int64))
    x = np.asarray(x, np.float32)
    batch = np.asarray(batch, np.int64)

    shared = {
        "w1": _ext_weights(np.asarray(W1, np.float32), np.asarray(a1s, np.float32),
                           np.asarray(a1d, np.float32), perm_rows=False),
        "w2": _ext_weights(np.asarray(W2, np.float32), np.asarray(a2s, np.float32),
                           np.asarray(a2d, np.float32), perm_rows=True),
        "w3": _ext_weights(np.asarray(W3, np.float32), np.asarray(a3s, np.float32),
                           np.asarray(a3d, np.float32), perm_rows=True),
        "b1r": np.tile(np.asarray(b1, np.float32)[_PERM_CM][None, :],
                       (P, 1)).astype(np_bf16),
        "b2r": np.tile(np.asarray(b2, np.float32)[_PERM_CM][None, :],
                       (P, 1)).astype(np_bf16),
        "b3r": np.tile(np.asarray(b3, np.float32)[None, :], (P, 1)).astype(np_bf16),
        "fcw": np.asarray(fcw, np.float32),
        "fcbr": np.tile(np.asarray(fcb, np.float32)[None, :], (G, 1)),
        "invcnt": (1.0 / np.maximum(
            np.bincount(batch, minlength=G), 1.0)).astype(np.float32)[:, None],
    }

    in_maps = []
    for k in range(NCORES):
        xk = np.zeros((NSHP, F_IN), np_bf16)
        xk[:NSH] = x[k * NSH:(k + 1) * NSH].astype(np_bf16)
        pidx = np.full((NSHP,), PAD, np.float32)
        pidx[:NSH] = batch[k * NSH:(k + 1) * NSH]
        poolidx = np.ascontiguousarray(pidx.reshape(NT, P).T)  # [P, NT]
        in_maps.append({
            "x0": xk,
            "esrc": np.ascontiguousarray(esrcT[k]),
            "edst": np.ascontiguousarray(edstT[k]),
            "poolidx": poolidx,
            **shared,
        })
    return plan, in_maps


_CACHE = {}


def _get_nc(plan, repeat=1, probe=frozenset()):
    key = (tuple(plan), repeat, tuple(sorted(probe)))
    if key not in _CACHE:
        _CACHE[key] = _build_nc(plan, repeat, probe)
    return _CACHE[key]


def kernel(x, edge_index, batch, W1, a1s, a1d, b1, W2, a2s, a2d, b2,
           W3, a3s, a3d, b3, fcw, fcb, _trace=False, _results=None):
    plan, in_maps = _prepare_inputs(x, edge_index, batch, W1, a1s, a1d, b1,
                                    W2, a2s, a2d, b2, W3, a3s, a3d, b3, fcw, fcb)
    nc = _get_nc(plan)
    res = run_bass_kernel_spmd(nc, in_maps, core_ids=list(range(NCORES)),
                               trace=_trace)
    if _results is not None:
        _results.append(res)
    return res.results[0]["out"]

